# revision 1
# baseline (speedup 1.0000x reference)
"""2-layer LSTM (B=128, T=1024, H=256) + last-step LayerNorm on 8 trn2 cores.

Data-parallel over batch (16 rows/core). Per core, everything is kept in a
transposed layout (hidden/gate dims on partitions, batch on the free axis):

  - gates.T for a block of 8 timesteps live in one PSUM tile [128, 1024]
    (col = 128*chunk + 16*step_in_block + b). The input projection
    x @ Wih.T + bias for the whole block is matmul-accumulated into the
    tile first (Wih.T chunks stationary, x.T streaming; bias via a K=1
    matmul against a ones row). Each timestep's recurrent term then
    accumulates into its 16-column slice (Whh.T chunks stationary, h.T
    streaming), so no separate add is ever needed.
  - sigmoid/tanh read the PSUM gate slices (gate order re-packed to
    i,f,o,g on the host so one sigmoid covers i|f|o), the c/h updates are
    short [128, 32] vector ops, and h is written in bf16 directly where
    the next matmul streams it from.
  - layer 2 runs 8 steps behind layer 1 so its per-8-step x-projection
    (from layer 1's h history ring) is ready, and so PE/ACT/DVE work of
    the two layers overlaps.

Matmuls are bf16 (fp32 PSUM accumulate); c stays fp32. Final step: PE
transpose of h2 back to [16, 256], LayerNorm, DMA out.
"""
import sys

sys.path.insert(0, "/opt/trn_rl_repo")

import numpy as np
import ml_dtypes

import concourse.bass as bass
import concourse.mybir as mybir
import concourse.tile as tile

NUM_LAYERS = 2
H = 256
F = 256
B, T = 128, 1024
LN_EPS = 1e-5
N_CORES = 8
BL = B // N_CORES          # batch rows per core = 16
G4 = 4 * H                 # 1024 gate dims
NCH = G4 // 128            # 8 chunks of gate dims
SBLK = 8                   # timesteps per x-projection block
BF16 = mybir.dt.bfloat16
F32 = mybir.dt.float32

# gate reorder: torch (i,f,g,o) -> (i,f,o,g) so sigmoid covers one slab
PERM = np.concatenate([np.arange(0, 2 * H), np.arange(3 * H, 4 * H),
                       np.arange(2 * H, 3 * H)])


def _split_excess_waits(nc):
    """walrus in this container rejects instructions with >2 sem waits
    (CoreV3 setupSyncWait). Hoist excess waits onto NoOps just before."""
    for fn in nc.m.functions:
        for blk in fn.blocks:
            insts = list(blk.instructions)
            out, n_new = [], 0
            for inst in insts:
                si = inst.sync_info
                waits = list(si.on_wait) if si is not None else []
                if len(waits) > 1:
                    head, rest = waits[:-1], waits[-1:]
                    # chain NoOps, one wait each (1-wait-per-inst walrus limit)
                    for wt in head:
                        nop = mybir.InstNoOp(
                            name=f"{inst.name}-ws{n_new}",
                            engine=inst.engine,
                            ins=[], outs=[],
                            sync_info=mybir.SyncInfo(on_wait=[wt], on_update=[]),
                        )
                        n_new += 1
                        out.append(nop)
                    inst.sync_info = mybir.SyncInfo(
                        on_wait=rest, on_update=list(si.on_update))
                out.append(inst)
            if n_new:
                try:
                    blk.instructions = out
                except Exception:
                    blk.set_instructions(out)


def build(t_steps=T):
    nc = bass.Bass()
    TB = t_steps * BL
    xt_d = nc.dram_tensor("xt", [2, 128, TB], BF16, kind="ExternalInput")
    wih_d = nc.dram_tensor("wih", [NUM_LAYERS, 2, 128, G4], BF16, kind="ExternalInput")
    whh_d = nc.dram_tensor("whh", [NUM_LAYERS, 2, 128, G4], BF16, kind="ExternalInput")
    bias_d = nc.dram_tensor("bias", [NUM_LAYERS, 1, G4], BF16, kind="ExternalInput")
    ident_d = nc.dram_tensor("ident", [128, 128], F32, kind="ExternalInput")
    gam_d = nc.dram_tensor("gam", [BL, H], F32, kind="ExternalInput")
    bet_d = nc.dram_tensor("bet", [BL, H], F32, kind="ExternalInput")
    y_d = nc.dram_tensor("y", [BL, H], F32, kind="ExternalOutput")

    NB = t_steps // SBLK
    with tile.TileContext(nc) as tc:
        with (
            tc.tile_pool(name="wts", bufs=1) as wts,
            tc.tile_pool(name="state", bufs=1) as st,
            tc.tile_pool(name="work", bufs=4) as wk,
            tc.tile_pool(name="psum", bufs=2, space="PSUM") as ps,
        ):
            # resident tensors (partition dim first on every SBUF tile)
            xt = [wts.tile([128, TB], BF16, tag=f"xt{kw}", name=f"xt{kw}") for kw in (0, 1)]
            for kw in (0, 1):
                nc.sync.dma_start(xt[kw][:], xt_d[kw])
            wih = [[wts.tile([128, G4], BF16, tag=f"wih{l}{kw}", name=f"wih{l}{kw}") for kw in (0, 1)]
                   for l in range(NUM_LAYERS)]
            whh = [[wts.tile([128, G4], BF16, tag=f"whh{l}{kw}", name=f"whh{l}{kw}") for kw in (0, 1)]
                   for l in range(NUM_LAYERS)]
            bias = [wts.tile([1, G4], BF16, tag=f"bias{l}", name=f"bias{l}") for l in range(NUM_LAYERS)]
            for l in range(NUM_LAYERS):
                for kw in (0, 1):
                    nc.sync.dma_start(wih[l][kw][:], wih_d[l, kw])
                    nc.sync.dma_start(whh[l][kw][:], whh_d[l, kw])
                nc.sync.dma_start(bias[l][:], bias_d[l])
            ident = wts.tile([128, 128], F32, tag="ident", name="ident")
            nc.sync.dma_start(ident[:], ident_d[:])
            ones = wts.tile([1, 128], BF16, tag="ones", name="ones")
            nc.vector.memset(ones[:], 1.0)
            zb = wts.tile([128, 1], F32, tag="zb", name="zb")
            nc.vector.memset(zb[:], 0.0)
            eps16 = wts.tile([16, 1], F32, tag="eps16", name="eps16")
            nc.vector.memset(eps16[:], LN_EPS)

            # persistent state
            hist = st.tile([128, SBLK * 32], BF16, tag="hist", name="hist")   # layer-1 h ring
            h2 = st.tile([128, 32], BF16, tag="h2", name="h2")
            c1 = st.tile([128, 32], F32, tag="c1", name="c1")
            c2 = st.tile([128, 32], F32, tag="c2", name="c2")
            h2f = st.tile([128, 32], F32, tag="h2f", name="h2f")

            xp_cur = [None, None]   # current psum block tile per layer

            def xproj_block(l, k):
                """accumulate Wih_l @ x_l.T + bias for steps [8k, 8k+8) into psum."""
                xp = ps.tile([128, SBLK * 128], F32, tag=f"xp{l}", name=f"xp{l}")
                xp_cur[l] = xp
                if l == 0:
                    rhs = [xt[kw][:, k * SBLK * BL:(k + 1) * SBLK * BL] for kw in (0, 1)]
                else:
                    hv = hist[:].rearrange("p (s w) -> p s w", s=SBLK)
                    rhs = [hv[:, :, 0:BL], hv[:, :, BL:2 * BL]]
                for ch in range(NCH):
                    o = xp[:, ch * 128:(ch + 1) * 128]
                    for kw in (0, 1):
                        nc.tensor.matmul(
                            o, wih[l][kw][:, ch * 128:(ch + 1) * 128], rhs[kw],
                            start=(kw == 0 and ch % 4 == 0), stop=False,
                            skip_group_check=True)
                    nc.tensor.matmul(
                        o, bias[l][:, ch * 128:(ch + 1) * 128], ones[:],
                        start=False, stop=True, skip_group_check=True)

            def step(l, t):
                """one LSTM timestep in transposed layout."""
                k, s = t // SBLK, t % SBLK
                if s == 0:
                    xproj_block(l, k)
                xp = xp_cur[l]
                c_t = c1 if l == 0 else c2
                if t > 0:
                    if l == 0:
                        hsrc = hist[:, ((t - 1) % SBLK) * 32:((t - 1) % SBLK) * 32 + 32]
                    else:
                        hsrc = h2[:]
                    for ch in range(NCH):
                        o = xp[:, ch * 128 + 16 * s: ch * 128 + 16 * s + 16]
                        for kw in (0, 1):
                            nc.tensor.matmul(
                                o, whh[l][kw][:, ch * 128:(ch + 1) * 128],
                                hsrc[:, 16 * kw:16 * kw + 16],
                                start=False, stop=(kw == 1), skip_group_check=True)
                xpv = xp[:].rearrange("p (c s w) -> p c s w", c=NCH, s=SBLK)
                sig = wk.tile([128, 96], F32, tag="sig", name="sig")
                nc.scalar.activation(sig[:].rearrange("p (c w) -> p c w", c=6),
                                     xpv[:, 0:6, s, :],
                                     mybir.ActivationFunctionType.Sigmoid,
                                     bias=zb[:])
                tg = wk.tile([128, 32], F32, tag="tg", name="tg")
                nc.scalar.activation(tg[:].rearrange("p (c w) -> p c w", c=2),
                                     xpv[:, 6:8, s, :],
                                     mybir.ActivationFunctionType.Tanh,
                                     bias=zb[:])
                ig = wk.tile([128, 32], F32, tag="ig", name="ig")
                nc.vector.tensor_mul(ig[:], sig[:, 0:32], tg[:])
                if t > 0:
                    fc = wk.tile([128, 32], F32, tag="fc", name="fc")
                    nc.vector.tensor_mul(fc[:], sig[:, 32:64], c_t[:])
                    nc.vector.tensor_add(c_t[:], ig[:], fc[:])
                else:
                    nc.vector.tensor_copy(c_t[:], ig[:])
                tc_ = wk.tile([128, 32], F32, tag="tc", name="tc")
                nc.scalar.activation(tc_[:], c_t[:],
                                     mybir.ActivationFunctionType.Tanh,
                                     bias=zb[:])
                if l == 0:
                    hdst = hist[:, (t % SBLK) * 32:(t % SBLK) * 32 + 32]
                else:
                    hdst = h2[:]
                nc.vector.tensor_mul(hdst, sig[:, 64:96], tc_[:])
                if l == 1 and t == t_steps - 1:
                    nc.vector.tensor_mul(h2f[:], sig[:, 64:96], tc_[:])

            for w in range(t_steps + SBLK):
                if SBLK <= w:
                    step(1, w - SBLK)
                if w < t_steps:
                    step(0, w)

            # ---- LayerNorm over H on h2f (h2.T layout) -> y [16, 256]
            pt = ps.tile([16, 256], F32, tag="xp0", name="xp0")
            nc.tensor.transpose(pt[:, 0:128], h2f[:, 0:16], ident[:])
            nc.tensor.transpose(pt[:, 128:256], h2f[:, 16:32], ident[:])
            hb = wk.tile([16, 256], F32, tag="hb", name="hb")
            nc.vector.tensor_copy(hb[:], pt[:])
            dum = wk.tile([16, 256], F32, tag="dum", name="dum")
            acc = wk.tile([16, 1], F32, tag="acc", name="acc")
            nc.scalar.activation(dum[:], hb[:], mybir.ActivationFunctionType.Copy,
                                 accum_out=acc[:])
            mu = wk.tile([16, 1], F32, tag="mu", name="mu")
            nc.vector.tensor_scalar_mul(mu[:], acc[:], 1.0 / H)
            cen = wk.tile([16, 256], F32, tag="cen", name="cen")
            nc.vector.tensor_scalar_sub(cen[:], hb[:], mu[:])
            acc2 = wk.tile([16, 1], F32, tag="acc2", name="acc2")
            nc.scalar.activation(dum[:], cen[:], mybir.ActivationFunctionType.Square,
                                 bias=zb[0:16, :], accum_out=acc2[:])
            sd = wk.tile([16, 1], F32, tag="sd", name="sd")
            nc.scalar.activation(sd[:], acc2[:], mybir.ActivationFunctionType.Sqrt,
                                 scale=1.0 / H, bias=eps16[:])
            rstd = wk.tile([16, 1], F32, tag="rstd", name="rstd")
            nc.vector.reciprocal(rstd[:], sd[:])
            nrm = wk.tile([16, 256], F32, tag="nrm", name="nrm")
            nc.vector.tensor_scalar_mul(nrm[:], cen[:], rstd[:])
            gam = wk.tile([16, 256], F32, tag="gam", name="gam")
            nc.sync.dma_start(gam[:], gam_d[:])
            bet = wk.tile([16, 256], F32, tag="bet", name="bet")
            nc.sync.dma_start(bet[:], bet_d[:])
            nc.vector.tensor_mul(nrm[:], nrm[:], gam[:])
            out = wk.tile([16, 256], F32, tag="out", name="out")
            nc.vector.tensor_add(out[:], nrm[:], bet[:])
            nc.sync.dma_start(y_d[:], out[:])

    _split_excess_waits(nc)
    return nc


def prep_inputs(x, W_ih, W_hh, b_ih, b_hh, ln_gamma, ln_beta, t_steps=T):
    """host-side shard + transpose + cast. Returns per-core input dicts."""
    bf = ml_dtypes.bfloat16
    wih = np.ascontiguousarray(
        np.transpose(W_ih[:, PERM, :], (0, 2, 1))).reshape(NUM_LAYERS, 2, 128, G4)
    whh = np.ascontiguousarray(
        np.transpose(W_hh[:, PERM, :], (0, 2, 1))).reshape(NUM_LAYERS, 2, 128, G4)
    bias = (b_ih + b_hh)[:, PERM].reshape(NUM_LAYERS, 1, G4)
    ident = np.eye(128, dtype=np.float32)
    ins = []
    for cid in range(N_CORES):
        xs = x[cid * BL:(cid + 1) * BL, :t_steps, :]        # [16, t, 256]
        xtp = np.transpose(xs, (2, 1, 0)).reshape(F, t_steps * BL)  # [256, t*16]
        ins.append({
            "xt": np.ascontiguousarray(xtp.reshape(2, 128, t_steps * BL)).astype(bf),
            "wih": wih.astype(bf), "whh": whh.astype(bf),
            "bias": bias.astype(bf), "ident": ident,
            "gam": np.broadcast_to(ln_gamma, (BL, H)).astype(np.float32).copy(),
            "bet": np.broadcast_to(ln_beta, (BL, H)).astype(np.float32).copy(),
        })
    return ins


_CACHED = {}


def kernel(x, W_ih, W_hh, b_ih, b_hh, ln_gamma, ln_beta):
    from concourse.bass_utils import run_bass_kernel_spmd
    x = np.asarray(x, dtype=np.float32)
    ins = prep_inputs(np.asarray(x), np.asarray(W_ih), np.asarray(W_hh),
                      np.asarray(b_ih), np.asarray(b_hh),
                      np.asarray(ln_gamma), np.asarray(ln_beta))
    if "nc" not in _CACHED:
        _CACHED["nc"] = build(T)
    res = run_bass_kernel_spmd(_CACHED["nc"], ins, core_ids=list(range(N_CORES)))
    return np.concatenate([res.results[c]["y"] for c in range(N_CORES)], axis=0)



# revision 3
# speedup vs baseline: 1.1949x; 1.1949x over previous
"""2-layer LSTM (B=128, T=1024, H=256) + last-step LayerNorm on 8 trn2 cores.

Data-parallel over batch (16 rows/core). Per core, everything is kept in a
transposed layout (hidden/gate dims on partitions, batch on the free axis):

  - gates.T for a block of 8 timesteps live in one PSUM tile [128, 1024] per
    layer (col = 128*chunk + 16*step_in_block + b). Each block's tile is
    initialized by two N=512 "indicator" matmuls that write the bias to the
    whole bank (start=True clears has_written), then the x-projection
    (Wih.T chunks stationary, x.T streaming) and per-step recurrent terms
    (Whh.T chunks stationary fp8, h.T streaming) accumulate on top.
  - the per-step chain is kept short: ONE sigmoid per layer covers all four
    gates (g rows are pre-scaled by 2 on the host so tanh(g) = 2*sig(2g)-1;
    the affine is a single fused tensor_scalar on DVE), then c/h updates are
    short [128, 32] vector ops, then tanh(c) and the h write (bf16, where
    the next matmul streams it from).
  - layer 2 runs 8 steps behind layer 1; per-wave issue order anti-phases
    the two chains across the PE/ACT/DVE FIFOs so one chain's matmul burst
    overlaps the other's activation/vector stages.
  - Whh is fp8 e4m3 (validated: final rel err ~9e-3 vs fp32 reference, vs
    ~3e-3 all-bf16); Wih/x/h stay bf16, accumulation fp32, c stays fp32.

Final step: PE transpose of h2 back to [16, 256], LayerNorm, DMA out.
"""
import sys

sys.path.insert(0, "/opt/trn_rl_repo")

import numpy as np
import ml_dtypes

import concourse.bass as bass
import concourse.mybir as mybir
import concourse.tile as tile

NUM_LAYERS = 2
H = 256
F = 256
B, T = 128, 1024
LN_EPS = 1e-5
N_CORES = 8
BL = B // N_CORES          # batch rows per core = 16
G4 = 4 * H                 # 1024 gate dims
NCH = G4 // 128            # 8 chunks of gate dims
SBLK = 8                   # timesteps per x-projection block
BF16 = mybir.dt.bfloat16
FP8 = mybir.dt.float8e4
F32 = mybir.dt.float32

# gate reorder: torch (i,f,g,o) -> (i,f,o,g)
PERM = np.concatenate([np.arange(0, 2 * H), np.arange(3 * H, 4 * H),
                       np.arange(2 * H, 3 * H)])


def _split_excess_waits(nc):
    """walrus in this container rejects instructions with >1 sem wait
    (CoreV3 setupSyncWait). Hoist excess waits onto NoOps just before."""
    for fn in nc.m.functions:
        for blk in fn.blocks:
            insts = list(blk.instructions)
            out, n_new = [], 0
            for inst in insts:
                si = inst.sync_info
                waits = list(si.on_wait) if si is not None else []
                if len(waits) > 1:
                    head, rest = waits[:-1], waits[-1:]
                    for wt in head:
                        nop = mybir.InstNoOp(
                            name=f"{inst.name}-ws{n_new}",
                            engine=inst.engine,
                            ins=[], outs=[],
                            sync_info=mybir.SyncInfo(on_wait=[wt], on_update=[]),
                        )
                        n_new += 1
                        out.append(nop)
                    inst.sync_info = mybir.SyncInfo(
                        on_wait=rest, on_update=list(si.on_update))
                out.append(inst)
            if n_new:
                try:
                    blk.instructions = out
                except Exception:
                    blk.set_instructions(out)


def build(t_steps=T):
    nc = bass.Bass()
    TB = t_steps * BL
    xt_d = nc.dram_tensor("xt", [2, 128, TB], BF16, kind="ExternalInput")
    wih_d = nc.dram_tensor("wih", [NUM_LAYERS, 2, 128, G4], BF16, kind="ExternalInput")
    whh_d = nc.dram_tensor("whh", [NUM_LAYERS, 2, 128, G4], FP8, kind="ExternalInput")
    biasm_d = nc.dram_tensor("biasm", [NUM_LAYERS, 2, 4, 128], BF16, kind="ExternalInput")
    ind_d = nc.dram_tensor("ind", [4, 512], BF16, kind="ExternalInput")
    ident_d = nc.dram_tensor("ident", [128, 128], F32, kind="ExternalInput")
    gam_d = nc.dram_tensor("gam", [BL, H], F32, kind="ExternalInput")
    bet_d = nc.dram_tensor("bet", [BL, H], F32, kind="ExternalInput")
    y_d = nc.dram_tensor("y", [BL, H], F32, kind="ExternalOutput")

    NB = t_steps // SBLK
    with tile.TileContext(nc) as tc:
        with (
            tc.tile_pool(name="wts", bufs=1) as wts,
            tc.tile_pool(name="state", bufs=1) as st,
            tc.tile_pool(name="work", bufs=4) as wk,
            tc.tile_pool(name="psum", bufs=2, space="PSUM") as ps,
        ):
            # resident tensors (partition dim first on every SBUF tile)
            xt = [wts.tile([128, TB], BF16, tag=f"xt{kw}", name=f"xt{kw}") for kw in (0, 1)]
            for kw in (0, 1):
                nc.sync.dma_start(xt[kw][:], xt_d[kw])
            wih = [[wts.tile([128, G4], BF16, tag=f"wih{l}{kw}", name=f"wih{l}{kw}") for kw in (0, 1)]
                   for l in range(NUM_LAYERS)]
            whh = [[wts.tile([128, G4], FP8, tag=f"whh{l}{kw}", name=f"whh{l}{kw}") for kw in (0, 1)]
                   for l in range(NUM_LAYERS)]
            biasm = [[wts.tile([4, 128], BF16, tag=f"bm{l}{b}", name=f"bm{l}{b}") for b in (0, 1)]
                     for l in range(NUM_LAYERS)]
            for l in range(NUM_LAYERS):
                for kw in (0, 1):
                    nc.sync.dma_start(wih[l][kw][:], wih_d[l, kw])
                    nc.sync.dma_start(whh[l][kw][:], whh_d[l, kw])
                for b in (0, 1):
                    nc.sync.dma_start(biasm[l][b][:], biasm_d[l, b])
            ind = wts.tile([4, 512], BF16, tag="ind", name="ind")
            nc.sync.dma_start(ind[:], ind_d[:])
            ident = wts.tile([128, 128], F32, tag="ident", name="ident")
            nc.sync.dma_start(ident[:], ident_d[:])
            zb = wts.tile([128, 1], F32, tag="zb", name="zb")
            nc.vector.memset(zb[:], 0.0)
            eps16 = wts.tile([16, 1], F32, tag="eps16", name="eps16")
            nc.vector.memset(eps16[:], LN_EPS)

            # persistent state
            hist = st.tile([128, SBLK * 32], BF16, tag="hist", name="hist")   # layer-1 h ring
            h2 = st.tile([128, 32], BF16, tag="h2", name="h2")
            c1 = st.tile([128, 32], F32, tag="c1", name="c1")
            c2 = st.tile([128, 32], F32, tag="c2", name="c2")
            h2f = st.tile([128, 32], F32, tag="h2f", name="h2f")

            xp_cur = [None, None]   # current psum block tile per layer
            xp_next0 = [None]       # layer-0 tile being prepped for next block

            def new_tile(l):
                return ps.tile([128, NCH * 128], F32, tag=f"xp{l}", name=f"xp{l}")

            def bias_mms(l, xp):
                for b in (0, 1):
                    nc.tensor.matmul(
                        xp[:, b * 512:(b + 1) * 512], biasm[l][b][:], ind[:],
                        start=True, stop=False, skip_group_check=True)

            def xproj_mms(l, k, chunks):
                """x-projection matmuls for block k of layer l, given chunk list."""
                xp = xp_cur[l] if l == 1 else xp_next0[0]
                if l == 0:
                    rhs = [xt[kw][:, k * SBLK * BL:(k + 1) * SBLK * BL] for kw in (0, 1)]
                else:
                    hv = hist[:].rearrange("p (s w) -> p s w", s=SBLK)
                    rhs = [hv[:, :, 0:BL], hv[:, :, BL:2 * BL]]
                for ch in chunks:
                    o = xp[:, ch * 128:(ch + 1) * 128]
                    for kw in (0, 1):
                        nc.tensor.matmul(
                            o, wih[l][kw][:, ch * 128:(ch + 1) * 128], rhs[kw],
                            start=False, stop=False, skip_group_check=True)

            def rec_mms(l, t):
                """recurrent matmuls for one step (all 8 chunks x 2 kw)."""
                if t == 0:
                    return
                s = t % SBLK
                xp = xp_cur[l]
                if l == 0:
                    hsrc = hist[:, ((t - 1) % SBLK) * 32:((t - 1) % SBLK) * 32 + 32]
                else:
                    hsrc = h2[:]
                for ch in range(NCH):
                    o = xp[:, ch * 128 + 16 * s: ch * 128 + 16 * s + 16]
                    for kw in (0, 1):
                        nc.tensor.matmul(
                            o, whh[l][kw][:, ch * 128:(ch + 1) * 128],
                            hsrc[:, 16 * kw:16 * kw + 16],
                            start=False, stop=(kw == 1), skip_group_check=True)

            sig_t = [None, None]
            tg_t = [None, None]
            ig_t = [None, None]
            fc_t = [None, None]
            tc_t = [None, None]

            def act_sig(l, t):
                """one sigmoid over all 4 gates (g pre-scaled x2 on host)."""
                s = t % SBLK
                xpv = xp_cur[l][:].rearrange("p (c s w) -> p c s w", c=NCH, s=SBLK)
                sig = wk.tile([128, 128], F32, tag=f"sig{l}", name=f"sig{l}")
                sig_t[l] = sig
                nc.scalar.activation(sig[:].rearrange("p (c w) -> p c w", c=NCH),
                                     xpv[:, 0:NCH, s, :],
                                     mybir.ActivationFunctionType.Sigmoid,
                                     bias=zb[:])

            def dve_c(l, t):
                """tanh(g) affine + c update."""
                sig = sig_t[l]
                c_t = c1 if l == 0 else c2
                if t > 0:
                    fc = wk.tile([128, 32], F32, tag=f"fc{l}", name=f"fc{l}")
                    fc_t[l] = fc
                    nc.vector.tensor_mul(fc[:], sig[:, 32:64], c_t[:])
                tg = wk.tile([128, 32], F32, tag=f"tg{l}", name=f"tg{l}")
                tg_t[l] = tg
                nc.vector.tensor_scalar(tg[:], sig[:, 96:128], 2.0, 1.0,
                                        mybir.AluOpType.mult,
                                        mybir.AluOpType.subtract)
                ig = wk.tile([128, 32], F32, tag=f"ig{l}", name=f"ig{l}")
                ig_t[l] = ig
                nc.vector.tensor_mul(ig[:], sig[:, 0:32], tg[:])
                if t > 0:
                    nc.vector.tensor_add(c_t[:], ig[:], fc_t[l][:])
                else:
                    nc.vector.tensor_copy(c_t[:], ig[:])

            def act_tc(l, t):
                c_t = c1 if l == 0 else c2
                tc_ = wk.tile([128, 32], F32, tag=f"tc{l}", name=f"tc{l}")
                tc_t[l] = tc_
                nc.scalar.activation(tc_[:], c_t[:],
                                     mybir.ActivationFunctionType.Tanh,
                                     bias=zb[:])

            def dve_h(l, t):
                sig = sig_t[l]
                if l == 0:
                    hdst = hist[:, (t % SBLK) * 32:(t % SBLK) * 32 + 32]
                else:
                    hdst = h2[:]
                nc.vector.tensor_mul(hdst, sig[:, 64:96], tc_t[l][:])
                if l == 1 and t == t_steps - 1:
                    nc.vector.tensor_mul(h2f[:], sig[:, 64:96], tc_t[l][:])

            # ---- main wave loop: wave w runs l0 step w and l1 step w-8
            for w in range(t_steps + SBLK):
                t0 = w if w < t_steps else None                  # layer-0 step
                t1 = w - SBLK if w >= SBLK else None             # layer-1 step
                s = w % SBLK

                # block-boundary prep (issue first so PE drains it before the
                # chains' recurrent bursts need the new tiles)
                if s == 0:
                    if t1 is not None:
                        # layer-1 tile for its new block: bias + xproj (source
                        # hist ring is complete exactly now)
                        xp_cur[1] = new_tile(1)
                        bias_mms(1, xp_cur[1])
                        xproj_mms(1, t1 // SBLK, range(NCH))
                    if t0 is not None:
                        if w == 0:
                            xp_next0[0] = new_tile(0)
                            bias_mms(0, xp_next0[0])
                            xproj_mms(0, 0, range(NCH))
                        xp_cur[0] = xp_next0[0]
                        xp_next0[0] = None

                # recurrent bursts: l1 first (its chain leads the wave)
                if t1 is not None:
                    rec_mms(1, t1)
                if t0 is not None:
                    rec_mms(0, t0)

                # spread layer-0 prep for the NEXT block over waves s=1..4
                # (issued after the chain-critical bursts; PE drains it in the
                # idle window while ACT/DVE run the post stages)
                if t0 is not None and t0 // SBLK + 1 < NB:
                    nk = t0 // SBLK + 1
                    if s == 1:
                        xp_next0[0] = new_tile(0)
                        bias_mms(0, xp_next0[0])
                        xproj_mms(0, nk, (0, 1))
                    elif s == 2:
                        xproj_mms(0, nk, (2, 3))
                    elif s == 3:
                        xproj_mms(0, nk, (4, 5))
                    elif s == 4:
                        xproj_mms(0, nk, (6, 7))

                # anti-phased post-matmul stages
                if t1 is not None:
                    act_sig(1, t1)
                    dve_c(1, t1)
                if t0 is not None:
                    act_sig(0, t0)
                if t1 is not None:
                    act_tc(1, t1)
                if t0 is not None:
                    dve_c(0, t0)
                if t1 is not None:
                    dve_h(1, t1)
                if t0 is not None:
                    act_tc(0, t0)
                    dve_h(0, t0)

            # ---- LayerNorm over H on h2f (h2.T layout) -> y [16, 256]
            pt = ps.tile([16, 256], F32, tag="xp0", name="pt")
            nc.tensor.transpose(pt[:, 0:128], h2f[:, 0:16], ident[:])
            nc.tensor.transpose(pt[:, 128:256], h2f[:, 16:32], ident[:])
            hb = wk.tile([16, 256], F32, tag="hb", name="hb")
            nc.vector.tensor_copy(hb[:], pt[:])
            dum = wk.tile([16, 256], F32, tag="dum", name="dum")
            acc = wk.tile([16, 1], F32, tag="acc", name="acc")
            nc.scalar.activation(dum[:], hb[:], mybir.ActivationFunctionType.Copy,
                                 accum_out=acc[:])
            mu = wk.tile([16, 1], F32, tag="mu", name="mu")
            nc.vector.tensor_scalar_mul(mu[:], acc[:], 1.0 / H)
            cen = wk.tile([16, 256], F32, tag="cen", name="cen")
            nc.vector.tensor_scalar_sub(cen[:], hb[:], mu[:])
            acc2 = wk.tile([16, 1], F32, tag="acc2", name="acc2")
            nc.scalar.activation(dum[:], cen[:], mybir.ActivationFunctionType.Square,
                                 bias=zb[0:16, :], accum_out=acc2[:])
            sd = wk.tile([16, 1], F32, tag="sd", name="sd")
            nc.scalar.activation(sd[:], acc2[:], mybir.ActivationFunctionType.Sqrt,
                                 scale=1.0 / H, bias=eps16[:])
            rstd = wk.tile([16, 1], F32, tag="rstd", name="rstd")
            nc.vector.reciprocal(rstd[:], sd[:])
            nrm = wk.tile([16, 256], F32, tag="nrm", name="nrm")
            nc.vector.tensor_scalar_mul(nrm[:], cen[:], rstd[:])
            gam = wk.tile([16, 256], F32, tag="gam", name="gam")
            nc.sync.dma_start(gam[:], gam_d[:])
            bet = wk.tile([16, 256], F32, tag="bet", name="bet")
            nc.sync.dma_start(bet[:], bet_d[:])
            nc.vector.tensor_mul(nrm[:], nrm[:], gam[:])
            out = wk.tile([16, 256], F32, tag="out", name="out")
            nc.vector.tensor_add(out[:], nrm[:], bet[:])
            nc.sync.dma_start(y_d[:], out[:])

    _split_excess_waits(nc)
    return nc


def prep_inputs(x, W_ih, W_hh, b_ih, b_hh, ln_gamma, ln_beta, t_steps=T):
    """host-side shard + transpose + cast. Returns per-core input dicts."""
    bf = ml_dtypes.bfloat16
    f8 = ml_dtypes.float8_e4m3
    # scale the g-gate rows (PERM'd rows 768:1024) by 2: tanh(g)=2*sig(2g)-1
    gscale = np.ones((G4, 1), np.float32)
    gscale[3 * H:] = 2.0
    wih_p = W_ih[:, PERM, :] * gscale
    whh_p = W_hh[:, PERM, :] * gscale
    bias_p = (b_ih + b_hh)[:, PERM] * gscale[:, 0]
    wih = np.ascontiguousarray(np.transpose(wih_p, (0, 2, 1))).reshape(NUM_LAYERS, 2, 128, G4)
    whh = np.ascontiguousarray(np.transpose(whh_p, (0, 2, 1))).reshape(NUM_LAYERS, 2, 128, G4)
    biasm = bias_p.reshape(NUM_LAYERS, 2, 4, 128)
    ind = np.zeros((4, 512), np.float32)
    for k in range(4):
        ind[k, k * 128:(k + 1) * 128] = 1.0
    ident = np.eye(128, dtype=np.float32)
    ins = []
    for cid in range(N_CORES):
        xs = x[cid * BL:(cid + 1) * BL, :t_steps, :]        # [16, t, 256]
        xtp = np.transpose(xs, (2, 1, 0)).reshape(F, t_steps * BL)  # [256, t*16]
        ins.append({
            "xt": np.ascontiguousarray(xtp.reshape(2, 128, t_steps * BL)).astype(bf),
            "wih": wih.astype(bf), "whh": whh.astype(f8),
            "biasm": biasm.astype(bf), "ind": ind.astype(bf),
            "ident": ident,
            "gam": np.broadcast_to(ln_gamma, (BL, H)).astype(np.float32).copy(),
            "bet": np.broadcast_to(ln_beta, (BL, H)).astype(np.float32).copy(),
        })
    return ins


_CACHED = {}


def kernel(x, W_ih, W_hh, b_ih, b_hh, ln_gamma, ln_beta):
    from concourse.bass_utils import run_bass_kernel_spmd
    x = np.asarray(x, dtype=np.float32)
    ins = prep_inputs(np.asarray(x), np.asarray(W_ih), np.asarray(W_hh),
                      np.asarray(b_ih), np.asarray(b_hh),
                      np.asarray(ln_gamma), np.asarray(ln_beta))
    if "nc" not in _CACHED:
        _CACHED["nc"] = build(T)
    res = run_bass_kernel_spmd(_CACHED["nc"], ins, core_ids=list(range(N_CORES)))
    return np.concatenate([res.results[c]["y"] for c in range(N_CORES)], axis=0)


# revision 9
# speedup vs baseline: 1.2949x; 1.0838x over previous
"""2-layer LSTM (B=128, T=1024, H=256) + last-step LayerNorm on 8 trn2 cores.

Data-parallel over batch (16 rows/core). Per core, everything is kept in a
transposed layout (hidden/gate dims on partitions, batch on the free axis):

  - gates.T for a block of 8 timesteps live in one PSUM tile [128, 1024] per
    layer (col = 128*chunk + 16*step_in_block + b). Each block's tile is
    initialized by two N=512 "indicator" matmuls that write the bias to the
    whole bank (start=True clears has_written), then the x-projection
    (Wih.T chunks stationary, x.T streaming) and per-step recurrent terms
    (Whh.T chunks stationary fp8, h.T streaming) accumulate on top.
  - the per-step chain is kept short: ONE sigmoid per layer covers all four
    gates (g rows are pre-scaled by 2 on the host so tanh(g) = 2*sig(2g)-1;
    the affine is a single fused tensor_scalar on DVE), then c/h updates are
    short [128, 32] vector ops, then tanh(c) and the h write (bf16, where
    the next matmul streams it from).
  - layer 2 runs 8 steps behind layer 1; per-wave issue order anti-phases
    the two chains across the PE/ACT/DVE FIFOs so one chain's matmul burst
    overlaps the other's activation/vector stages.
  - all operands bf16 (fp8 Whh was tried: no speedup -- the recurrent burst
    is MM-issue-floor-bound at ~27ns/tile either way -- so bf16 keeps the
    better numerics), accumulation fp32, c stays fp32.

Final step: PE transpose of h2 back to [16, 256], LayerNorm, DMA out.
"""
import sys

sys.path.insert(0, "/opt/trn_rl_repo")

import numpy as np
import ml_dtypes

import concourse.bass as bass
import concourse.mybir as mybir
import concourse.tile as tile

NUM_LAYERS = 2
H = 256
F = 256
B, T = 128, 1024
LN_EPS = 1e-5
N_CORES = 8
BL = B // N_CORES          # batch rows per core = 16
G4 = 4 * H                 # 1024 gate dims
NCH = G4 // 128            # 8 chunks of gate dims
SBLK = 8                   # timesteps per x-projection block
BF16 = mybir.dt.bfloat16
F32 = mybir.dt.float32

# gate reorder: torch (i,f,g,o) -> (i,f,o,g)
PERM = np.concatenate([np.arange(0, 2 * H), np.arange(3 * H, 4 * H),
                       np.arange(2 * H, 3 * H)])


def _split_excess_waits(nc):
    """walrus in this container rejects instructions with >1 sem wait
    (CoreV3 setupSyncWait). Hoist excess waits onto NoOps just before."""
    for fn in nc.m.functions:
        for blk in fn.blocks:
            insts = list(blk.instructions)
            out, n_new = [], 0
            for inst in insts:
                si = inst.sync_info
                waits = list(si.on_wait) if si is not None else []
                if len(waits) > 1:
                    head, rest = waits[:-1], waits[-1:]
                    for wt in head:
                        nop = mybir.InstNoOp(
                            name=f"{inst.name}-ws{n_new}",
                            engine=inst.engine,
                            ins=[], outs=[],
                            sync_info=mybir.SyncInfo(on_wait=[wt], on_update=[]),
                        )
                        n_new += 1
                        out.append(nop)
                    inst.sync_info = mybir.SyncInfo(
                        on_wait=rest, on_update=list(si.on_update))
                out.append(inst)
            if n_new:
                try:
                    blk.instructions = out
                except Exception:
                    blk.set_instructions(out)


def build(t_steps=T):
    nc = bass.Bass()
    TB = t_steps * BL
    xt_d = nc.dram_tensor("xt", [2, 128, TB], BF16, kind="ExternalInput")
    wih_d = nc.dram_tensor("wih", [NUM_LAYERS, 2, 128, G4], BF16, kind="ExternalInput")
    whh_d = nc.dram_tensor("whh", [NUM_LAYERS, 2, 128, G4], BF16, kind="ExternalInput")
    biasm_d = nc.dram_tensor("biasm", [NUM_LAYERS, 2, 4, 128], BF16, kind="ExternalInput")
    ind_d = nc.dram_tensor("ind", [4, 512], BF16, kind="ExternalInput")
    ident_d = nc.dram_tensor("ident", [128, 128], F32, kind="ExternalInput")
    gam_d = nc.dram_tensor("gam", [BL, H], F32, kind="ExternalInput")
    bet_d = nc.dram_tensor("bet", [BL, H], F32, kind="ExternalInput")
    y_d = nc.dram_tensor("y", [BL, H], F32, kind="ExternalOutput")

    NB = t_steps // SBLK
    with tile.TileContext(nc) as tc:
        with (
            tc.tile_pool(name="wts", bufs=1) as wts,
            tc.tile_pool(name="state", bufs=1) as st,
            tc.tile_pool(name="work", bufs=4) as wk,
            tc.tile_pool(name="psum", bufs=2, space="PSUM") as ps,
        ):
            # resident tensors (partition dim first on every SBUF tile)
            xt = [wts.tile([128, TB], BF16, tag=f"xt{kw}", name=f"xt{kw}") for kw in (0, 1)]
            for kw in (0, 1):
                nc.sync.dma_start(xt[kw][:], xt_d[kw])
            wih = [[wts.tile([128, G4], BF16, tag=f"wih{l}{kw}", name=f"wih{l}{kw}") for kw in (0, 1)]
                   for l in range(NUM_LAYERS)]
            whh = [[wts.tile([128, G4], BF16, tag=f"whh{l}{kw}", name=f"whh{l}{kw}") for kw in (0, 1)]
                   for l in range(NUM_LAYERS)]
            biasm = [[wts.tile([4, 128], BF16, tag=f"bm{l}{b}", name=f"bm{l}{b}") for b in (0, 1)]
                     for l in range(NUM_LAYERS)]
            for l in range(NUM_LAYERS):
                for kw in (0, 1):
                    nc.sync.dma_start(wih[l][kw][:], wih_d[l, kw])
                    nc.sync.dma_start(whh[l][kw][:], whh_d[l, kw])
                for b in (0, 1):
                    nc.sync.dma_start(biasm[l][b][:], biasm_d[l, b])
            ind = wts.tile([4, 512], BF16, tag="ind", name="ind")
            nc.sync.dma_start(ind[:], ind_d[:])
            ident = wts.tile([128, 128], F32, tag="ident", name="ident")
            nc.sync.dma_start(ident[:], ident_d[:])
            zb = wts.tile([128, 1], F32, tag="zb", name="zb")
            nc.vector.memset(zb[:], 0.0)
            eps16 = wts.tile([16, 1], F32, tag="eps16", name="eps16")
            nc.vector.memset(eps16[:], LN_EPS)

            # persistent state
            hist = st.tile([128, SBLK * 32], BF16, tag="hist", name="hist")   # layer-1 h ring
            h2 = st.tile([128, 32], BF16, tag="h2", name="h2")
            c1 = st.tile([128, 32], F32, tag="c1", name="c1")
            c2 = st.tile([128, 32], F32, tag="c2", name="c2")
            h2f = st.tile([128, 32], F32, tag="h2f", name="h2f")

            xp_cur = [None, None]   # current psum block tile per layer
            xp_next0 = [None]       # layer-0 tile being prepped for next block
            xp_next1 = [None]       # layer-1 tile being prepped for next block

            def new_tile(l):
                return ps.tile([128, NCH * 128], F32, tag=f"xp{l}", name=f"xp{l}")

            def bias_mms(l, xp):
                for b in (0, 1):
                    nc.tensor.matmul(
                        xp[:, b * 512:(b + 1) * 512], biasm[l][b][:], ind[:],
                        start=True, stop=False, skip_group_check=True)

            def xproj_mms(l, k, chunks):
                """x-projection matmuls (all slots) for block k of layer 0."""
                xp = xp_next0[0]
                rhs = [xt[kw][:, k * SBLK * BL:(k + 1) * SBLK * BL] for kw in (0, 1)]
                for ch in chunks:
                    o = xp[:, ch * 128:(ch + 1) * 128]
                    for kw in (0, 1):
                        nc.tensor.matmul(
                            o, wih[l][kw][:, ch * 128:(ch + 1) * 128], rhs[kw],
                            start=False, stop=False, skip_group_check=True)

            def xproj1_mms(xp, s0, s1):
                """layer-1 x-projection for ring slots [s0, s1) (all chunks)."""
                hv = hist[:].rearrange("p (s w) -> p s w", s=SBLK)
                rhs = [hv[:, s0:s1, 0:BL], hv[:, s0:s1, BL:2 * BL]]
                for ch in range(NCH):
                    o = xp[:, ch * 128 + s0 * 16: ch * 128 + s1 * 16]
                    for kw in (0, 1):
                        nc.tensor.matmul(
                            o, wih[1][kw][:, ch * 128:(ch + 1) * 128], rhs[kw],
                            start=False, stop=False, skip_group_check=True)

            def rec_mms(l, t):
                """recurrent matmuls for one step (all 8 chunks x 2 kw)."""
                if t == 0:
                    return
                s = t % SBLK
                xp = xp_cur[l]
                if l == 0:
                    hsrc = hist[:, ((t - 1) % SBLK) * 32:((t - 1) % SBLK) * 32 + 32]
                else:
                    hsrc = h2[:]
                for ch in range(NCH):
                    o = xp[:, ch * 128 + 16 * s: ch * 128 + 16 * s + 16]
                    for kw in (0, 1):
                        nc.tensor.matmul(
                            o, whh[l][kw][:, ch * 128:(ch + 1) * 128],
                            hsrc[:, 16 * kw:16 * kw + 16],
                            start=False, stop=(kw == 1), skip_group_check=True)

            sig_t = [None, None]
            tg_t = [None, None]
            ig_t = [None, None]
            fc_t = [None, None]
            tc_t = [None, None]

            def act_sig(l, t):
                """one sigmoid over all 4 gates (g pre-scaled x2 on host)."""
                s = t % SBLK
                xpv = xp_cur[l][:].rearrange("p (c s w) -> p c s w", c=NCH, s=SBLK)
                sig = wk.tile([128, 128], F32, tag=f"sig{l}", name=f"sig{l}")
                sig_t[l] = sig
                nc.scalar.activation(sig[:].rearrange("p (c w) -> p c w", c=NCH),
                                     xpv[:, 0:NCH, s, :],
                                     mybir.ActivationFunctionType.Sigmoid,
                                     bias=zb[:])

            def dve_c(l, t):
                """tanh(g) affine + c update."""
                sig = sig_t[l]
                c_t = c1 if l == 0 else c2
                tg = wk.tile([128, 32], F32, tag=f"tg{l}", name=f"tg{l}")
                tg_t[l] = tg
                nc.vector.tensor_scalar(tg[:], sig[:, 96:128], 2.0, 1.0,
                                        mybir.AluOpType.mult,
                                        mybir.AluOpType.subtract)
                if t > 0:
                    fc = wk.tile([128, 32], F32, tag=f"fc{l}", name=f"fc{l}")
                    fc_t[l] = fc
                    nc.vector.tensor_mul(fc[:], sig[:, 32:64], c_t[:])
                ig = wk.tile([128, 32], F32, tag=f"ig{l}", name=f"ig{l}")
                ig_t[l] = ig
                nc.vector.tensor_mul(ig[:], sig[:, 0:32], tg[:])
                if t > 0:
                    nc.vector.tensor_add(c_t[:], ig[:], fc_t[l][:])
                else:
                    nc.vector.tensor_copy(c_t[:], ig[:])

            def act_tc(l, t):
                c_t = c1 if l == 0 else c2
                tc_ = wk.tile([128, 32], F32, tag=f"tc{l}", name=f"tc{l}")
                tc_t[l] = tc_
                nc.scalar.activation(tc_[:], c_t[:],
                                     mybir.ActivationFunctionType.Tanh,
                                     bias=zb[:])

            def dve_h(l, t):
                sig = sig_t[l]
                if l == 0:
                    hdst = hist[:, (t % SBLK) * 32:(t % SBLK) * 32 + 32]
                else:
                    hdst = h2[:]
                nc.vector.tensor_mul(hdst, sig[:, 64:96], tc_t[l][:])
                if l == 1 and t == t_steps - 1:
                    nc.vector.tensor_mul(h2f[:], sig[:, 64:96], tc_t[l][:])

            # ---- main wave loop: wave w runs l0 step w and l1 step w-8
            for w in range(t_steps + SBLK):
                t0 = w if w < t_steps else None                  # layer-0 step
                t1 = w - SBLK if w >= SBLK else None             # layer-1 step
                s = w % SBLK

                # block-boundary tile switches (prep happened in prior waves)
                if s == 0:
                    if t1 is not None:
                        xp_cur[1] = xp_next1[0]
                        xp_next1[0] = None
                    if t0 is not None:
                        if w == 0:
                            xp_next0[0] = new_tile(0)
                            bias_mms(0, xp_next0[0])
                            xproj_mms(0, 0, range(NCH))
                        xp_cur[0] = xp_next0[0]
                        xp_next0[0] = None

                # recurrent bursts: l1 first (its chain leads the wave)
                if t1 is not None:
                    rec_mms(1, t1)
                if t0 is not None:
                    rec_mms(0, t0)

                # prep work, issued after the chain-critical bursts; PE drains
                # it in the idle window while ACT/DVE run the post stages.
                # layer-1 xproj for ring slots {6,7} of the block starting now
                # (slot 7 was written only at the end of the previous wave):
                if s == 0 and t1 is not None:
                    xproj1_mms(xp_cur[1], 6, 8)
                # layer-1 tile for the next block: bias at s=1, then xproj of
                # ring-slot pairs as soon as each pair of h1 values lands
                k1 = w // SBLK
                if k1 < NB:
                    if s == 1:
                        xp_next1[0] = new_tile(1)
                        bias_mms(1, xp_next1[0])
                    elif s == 2:
                        xproj1_mms(xp_next1[0], 0, 2)
                    elif s == 4:
                        xproj1_mms(xp_next1[0], 2, 4)
                    elif s == 6:
                        xproj1_mms(xp_next1[0], 4, 6)
                # layer-0 prep for the next block (x is in SBUF; free order)
                if t0 is not None and t0 // SBLK + 1 < NB:
                    nk = t0 // SBLK + 1
                    if s == 1:
                        xp_next0[0] = new_tile(0)
                        bias_mms(0, xp_next0[0])
                        xproj_mms(0, nk, (0, 1))
                    elif s == 2:
                        xproj_mms(0, nk, (2, 3))
                    elif s == 3:
                        xproj_mms(0, nk, (4, 5))
                    elif s == 4:
                        xproj_mms(0, nk, (6, 7))

                # anti-phased post-matmul stages
                if t1 is not None:
                    act_sig(1, t1)
                    dve_c(1, t1)
                if t0 is not None:
                    act_sig(0, t0)
                if t1 is not None:
                    act_tc(1, t1)
                if t0 is not None:
                    dve_c(0, t0)
                if t1 is not None:
                    dve_h(1, t1)
                if t0 is not None:
                    act_tc(0, t0)
                    dve_h(0, t0)

            # ---- LayerNorm over H on h2f (h2.T layout) -> y [16, 256]
            pt = ps.tile([16, 256], F32, tag="xp0", name="pt")
            nc.tensor.transpose(pt[:, 0:128], h2f[:, 0:16], ident[:])
            nc.tensor.transpose(pt[:, 128:256], h2f[:, 16:32], ident[:])
            hb = wk.tile([16, 256], F32, tag="hb", name="hb")
            nc.vector.tensor_copy(hb[:], pt[:])
            dum = wk.tile([16, 256], F32, tag="dum", name="dum")
            acc = wk.tile([16, 1], F32, tag="acc", name="acc")
            nc.scalar.activation(dum[:], hb[:], mybir.ActivationFunctionType.Copy,
                                 accum_out=acc[:])
            mu = wk.tile([16, 1], F32, tag="mu", name="mu")
            nc.vector.tensor_scalar_mul(mu[:], acc[:], 1.0 / H)
            cen = wk.tile([16, 256], F32, tag="cen", name="cen")
            nc.vector.tensor_scalar_sub(cen[:], hb[:], mu[:])
            acc2 = wk.tile([16, 1], F32, tag="acc2", name="acc2")
            nc.scalar.activation(dum[:], cen[:], mybir.ActivationFunctionType.Square,
                                 bias=zb[0:16, :], accum_out=acc2[:])
            sd = wk.tile([16, 1], F32, tag="sd", name="sd")
            nc.scalar.activation(sd[:], acc2[:], mybir.ActivationFunctionType.Sqrt,
                                 scale=1.0 / H, bias=eps16[:])
            rstd = wk.tile([16, 1], F32, tag="rstd", name="rstd")
            nc.vector.reciprocal(rstd[:], sd[:])
            nrm = wk.tile([16, 256], F32, tag="nrm", name="nrm")
            nc.vector.tensor_scalar_mul(nrm[:], cen[:], rstd[:])
            gam = wk.tile([16, 256], F32, tag="gam", name="gam")
            nc.sync.dma_start(gam[:], gam_d[:])
            bet = wk.tile([16, 256], F32, tag="bet", name="bet")
            nc.sync.dma_start(bet[:], bet_d[:])
            nc.vector.tensor_mul(nrm[:], nrm[:], gam[:])
            out = wk.tile([16, 256], F32, tag="out", name="out")
            nc.vector.tensor_add(out[:], nrm[:], bet[:])
            nc.sync.dma_start(y_d[:], out[:])

    _split_excess_waits(nc)
    return nc


def prep_inputs(x, W_ih, W_hh, b_ih, b_hh, ln_gamma, ln_beta, t_steps=T):
    """host-side shard + transpose + cast. Returns per-core input dicts."""
    bf = ml_dtypes.bfloat16
    # scale the g-gate rows (PERM'd rows 768:1024) by 2: tanh(g)=2*sig(2g)-1
    gscale = np.ones((G4, 1), np.float32)
    gscale[3 * H:] = 2.0
    wih_p = W_ih[:, PERM, :] * gscale
    whh_p = W_hh[:, PERM, :] * gscale
    bias_p = (b_ih + b_hh)[:, PERM] * gscale[:, 0]
    wih = np.ascontiguousarray(np.transpose(wih_p, (0, 2, 1))).reshape(NUM_LAYERS, 2, 128, G4)
    whh = np.ascontiguousarray(np.transpose(whh_p, (0, 2, 1))).reshape(NUM_LAYERS, 2, 128, G4)
    biasm = bias_p.reshape(NUM_LAYERS, 2, 4, 128)
    ind = np.zeros((4, 512), np.float32)
    for k in range(4):
        ind[k, k * 128:(k + 1) * 128] = 1.0
    ident = np.eye(128, dtype=np.float32)
    ins = []
    for cid in range(N_CORES):
        xs = x[cid * BL:(cid + 1) * BL, :t_steps, :]        # [16, t, 256]
        xtp = np.transpose(xs, (2, 1, 0)).reshape(F, t_steps * BL)  # [256, t*16]
        ins.append({
            "xt": np.ascontiguousarray(xtp.reshape(2, 128, t_steps * BL)).astype(bf),
            "wih": wih.astype(bf), "whh": whh.astype(bf),
            "biasm": biasm.astype(bf), "ind": ind.astype(bf),
            "ident": ident,
            "gam": np.broadcast_to(ln_gamma, (BL, H)).astype(np.float32).copy(),
            "bet": np.broadcast_to(ln_beta, (BL, H)).astype(np.float32).copy(),
        })
    return ins


_CACHED = {}


def kernel(x, W_ih, W_hh, b_ih, b_hh, ln_gamma, ln_beta):
    from concourse.bass_utils import run_bass_kernel_spmd
    x = np.asarray(x, dtype=np.float32)
    ins = prep_inputs(np.asarray(x), np.asarray(W_ih), np.asarray(W_hh),
                      np.asarray(b_ih), np.asarray(b_hh),
                      np.asarray(ln_gamma), np.asarray(ln_beta))
    if "nc" not in _CACHED:
        _CACHED["nc"] = build(T)
    res = run_bass_kernel_spmd(_CACHED["nc"], ins, core_ids=list(range(N_CORES)))
    return np.concatenate([res.results[c]["y"] for c in range(N_CORES)], axis=0)


# revision 10
# speedup vs baseline: 1.2951x; 1.0001x over previous
"""2-layer LSTM (B=128, T=1024, H=256) + last-step LayerNorm on 8 trn2 cores.

Data-parallel over batch (16 rows/core). Per core, everything is kept in a
transposed layout (hidden/gate dims on partitions, batch on the free axis):

  - gates.T for a block of 8 timesteps live in one PSUM tile [128, 1024] per
    layer (col = 128*chunk + 16*step_in_block + b). Each block's tile is
    initialized by two N=512 "indicator" matmuls that write the bias to the
    whole bank (start=True clears has_written), then the x-projection
    (Wih.T chunks stationary, x.T streaming) and per-step recurrent terms
    (Whh.T chunks stationary fp8, h.T streaming) accumulate on top.
  - the per-step chain is kept short: ONE sigmoid per layer covers all four
    gates (g rows are pre-scaled by 2 on the host so tanh(g) = 2*sig(2g)-1;
    the affine is a single fused tensor_scalar on DVE), then c/h updates are
    short [128, 32] vector ops, then tanh(c) and the h write (bf16, where
    the next matmul streams it from).
  - layer 2 runs 8 steps behind layer 1; per-wave issue order anti-phases
    the two chains across the PE/ACT/DVE FIFOs so one chain's matmul burst
    overlaps the other's activation/vector stages.
  - all operands bf16 (fp8 Whh was tried: no speedup -- the recurrent burst
    is MM-issue-floor-bound at ~27ns/tile either way -- so bf16 keeps the
    better numerics), accumulation fp32, c stays fp32.

Final step: PE transpose of h2 back to [16, 256], LayerNorm, DMA out.
"""
import sys

sys.path.insert(0, "/opt/trn_rl_repo")

import numpy as np
import ml_dtypes

import concourse.bass as bass
import concourse.mybir as mybir
import concourse.tile as tile

NUM_LAYERS = 2
H = 256
F = 256
B, T = 128, 1024
LN_EPS = 1e-5
N_CORES = 8
BL = B // N_CORES          # batch rows per core = 16
G4 = 4 * H                 # 1024 gate dims
NCH = G4 // 128            # 8 chunks of gate dims
SBLK = 8                   # timesteps per x-projection block
BF16 = mybir.dt.bfloat16
F32 = mybir.dt.float32

# gate reorder: torch (i,f,g,o) -> (i,f,o,g)
PERM = np.concatenate([np.arange(0, 2 * H), np.arange(3 * H, 4 * H),
                       np.arange(2 * H, 3 * H)])


def _split_excess_waits(nc):
    """walrus in this container rejects instructions with >1 sem wait
    (CoreV3 setupSyncWait). Hoist excess waits onto NoOps just before."""
    for fn in nc.m.functions:
        for blk in fn.blocks:
            insts = list(blk.instructions)
            out, n_new = [], 0
            for inst in insts:
                si = inst.sync_info
                waits = list(si.on_wait) if si is not None else []
                if len(waits) > 1:
                    head, rest = waits[:-1], waits[-1:]
                    for wt in head:
                        nop = mybir.InstNoOp(
                            name=f"{inst.name}-ws{n_new}",
                            engine=inst.engine,
                            ins=[], outs=[],
                            sync_info=mybir.SyncInfo(on_wait=[wt], on_update=[]),
                        )
                        n_new += 1
                        out.append(nop)
                    inst.sync_info = mybir.SyncInfo(
                        on_wait=rest, on_update=list(si.on_update))
                out.append(inst)
            if n_new:
                try:
                    blk.instructions = out
                except Exception:
                    blk.set_instructions(out)


def build(t_steps=T):
    nc = bass.Bass()
    TB = t_steps * BL
    xt_d = nc.dram_tensor("xt", [2, 128, TB], BF16, kind="ExternalInput")
    wih_d = nc.dram_tensor("wih", [NUM_LAYERS, 2, 128, G4], BF16, kind="ExternalInput")
    whh_d = nc.dram_tensor("whh", [NUM_LAYERS, 2, 128, G4], BF16, kind="ExternalInput")
    biasm_d = nc.dram_tensor("biasm", [NUM_LAYERS, 2, 4, 128], BF16, kind="ExternalInput")
    ind_d = nc.dram_tensor("ind", [4, 512], BF16, kind="ExternalInput")
    ident_d = nc.dram_tensor("ident", [128, 128], F32, kind="ExternalInput")
    gam_d = nc.dram_tensor("gam", [BL, H], F32, kind="ExternalInput")
    bet_d = nc.dram_tensor("bet", [BL, H], F32, kind="ExternalInput")
    y_d = nc.dram_tensor("y", [BL, H], F32, kind="ExternalOutput")

    NB = t_steps // SBLK
    with tile.TileContext(nc) as tc:
        with (
            tc.tile_pool(name="wts", bufs=1) as wts,
            tc.tile_pool(name="state", bufs=1) as st,
            tc.tile_pool(name="work", bufs=4) as wk,
            tc.tile_pool(name="psum", bufs=2, space="PSUM") as ps,
        ):
            # resident tensors (partition dim first on every SBUF tile)
            xt = [wts.tile([128, TB], BF16, tag=f"xt{kw}", name=f"xt{kw}") for kw in (0, 1)]
            for kw in (0, 1):
                nc.sync.dma_start(xt[kw][:], xt_d[kw])
            wih = [[wts.tile([128, G4], BF16, tag=f"wih{l}{kw}", name=f"wih{l}{kw}") for kw in (0, 1)]
                   for l in range(NUM_LAYERS)]
            whh = [[wts.tile([128, G4], BF16, tag=f"whh{l}{kw}", name=f"whh{l}{kw}") for kw in (0, 1)]
                   for l in range(NUM_LAYERS)]
            biasm = [[wts.tile([4, 128], BF16, tag=f"bm{l}{b}", name=f"bm{l}{b}") for b in (0, 1)]
                     for l in range(NUM_LAYERS)]
            for l in range(NUM_LAYERS):
                for kw in (0, 1):
                    nc.sync.dma_start(wih[l][kw][:], wih_d[l, kw])
                    nc.sync.dma_start(whh[l][kw][:], whh_d[l, kw])
                for b in (0, 1):
                    nc.sync.dma_start(biasm[l][b][:], biasm_d[l, b])
            ind = wts.tile([4, 512], BF16, tag="ind", name="ind")
            nc.sync.dma_start(ind[:], ind_d[:])
            ident = wts.tile([128, 128], F32, tag="ident", name="ident")
            nc.sync.dma_start(ident[:], ident_d[:])
            zb = wts.tile([128, 1], F32, tag="zb", name="zb")
            nc.vector.memset(zb[:], 0.0)
            eps16 = wts.tile([16, 1], F32, tag="eps16", name="eps16")
            nc.vector.memset(eps16[:], LN_EPS)

            # persistent state
            hist = st.tile([128, SBLK * 32], BF16, tag="hist", name="hist")   # layer-1 h ring
            h2 = st.tile([128, 32], BF16, tag="h2", name="h2")
            c1 = st.tile([128, 32], F32, tag="c1", name="c1")
            c2 = st.tile([128, 32], F32, tag="c2", name="c2")
            h2f = st.tile([128, 32], F32, tag="h2f", name="h2f")

            xp_cur = [None, None]   # current psum block tile per layer
            xp_next0 = [None]       # layer-0 tile being prepped for next block
            xp_next1 = [None]       # layer-1 tile being prepped for next block

            def new_tile(l):
                return ps.tile([128, NCH * 128], F32, tag=f"xp{l}", name=f"xp{l}")

            def bias_mms(l, xp):
                for b in (0, 1):
                    nc.tensor.matmul(
                        xp[:, b * 512:(b + 1) * 512], biasm[l][b][:], ind[:],
                        start=True, stop=False, skip_group_check=True)

            def xproj_mms(l, k, chunks):
                """x-projection matmuls (all slots) for block k of layer 0."""
                xp = xp_next0[0]
                rhs = [xt[kw][:, k * SBLK * BL:(k + 1) * SBLK * BL] for kw in (0, 1)]
                for ch in chunks:
                    o = xp[:, ch * 128:(ch + 1) * 128]
                    for kw in (0, 1):
                        nc.tensor.matmul(
                            o, wih[l][kw][:, ch * 128:(ch + 1) * 128], rhs[kw],
                            start=False, stop=False, skip_group_check=True)

            def xproj1_mms(xp, s0, s1):
                """layer-1 x-projection for ring slots [s0, s1) (all chunks)."""
                hv = hist[:].rearrange("p (s w) -> p s w", s=SBLK)
                rhs = [hv[:, s0:s1, 0:BL], hv[:, s0:s1, BL:2 * BL]]
                for ch in range(NCH):
                    o = xp[:, ch * 128 + s0 * 16: ch * 128 + s1 * 16]
                    for kw in (0, 1):
                        nc.tensor.matmul(
                            o, wih[1][kw][:, ch * 128:(ch + 1) * 128], rhs[kw],
                            start=False, stop=False, skip_group_check=True)

            def rec_mms(l, t):
                """recurrent matmuls for one step (all 8 chunks x 2 kw)."""
                if t == 0:
                    return
                s = t % SBLK
                xp = xp_cur[l]
                if l == 0:
                    hsrc = hist[:, ((t - 1) % SBLK) * 32:((t - 1) % SBLK) * 32 + 32]
                else:
                    hsrc = h2[:]
                for ch in range(NCH):
                    o = xp[:, ch * 128 + 16 * s: ch * 128 + 16 * s + 16]
                    for kw in (0, 1):
                        nc.tensor.matmul(
                            o, whh[l][kw][:, ch * 128:(ch + 1) * 128],
                            hsrc[:, 16 * kw:16 * kw + 16],
                            start=False, stop=(kw == 1), skip_group_check=True)

            sig_t = [None, None]
            tg_t = [None, None]
            ig_t = [None, None]
            fc_t = [None, None]
            tc_t = [None, None]

            def act_sig(l, t):
                """one sigmoid over all 4 gates (g pre-scaled x2 on host)."""
                s = t % SBLK
                xpv = xp_cur[l][:].rearrange("p (c s w) -> p c s w", c=NCH, s=SBLK)
                sig = wk.tile([128, 128], F32, tag=f"sig{l}", name=f"sig{l}")
                sig_t[l] = sig
                nc.scalar.activation(sig[:].rearrange("p (c w) -> p c w", c=NCH),
                                     xpv[:, 0:NCH, s, :],
                                     mybir.ActivationFunctionType.Sigmoid,
                                     bias=zb[:])

            def dve_c(l, t):
                """tanh(g) affine + c update."""
                sig = sig_t[l]
                c_t = c1 if l == 0 else c2
                tg = wk.tile([128, 32], F32, tag=f"tg{l}", name=f"tg{l}")
                tg_t[l] = tg
                nc.vector.tensor_scalar(tg[:], sig[:, 96:128], 2.0, 1.0,
                                        mybir.AluOpType.mult,
                                        mybir.AluOpType.subtract)
                if t > 0:
                    fc = wk.tile([128, 32], F32, tag=f"fc{l}", name=f"fc{l}")
                    fc_t[l] = fc
                    nc.vector.tensor_mul(fc[:], sig[:, 32:64], c_t[:])
                ig = wk.tile([128, 32], F32, tag=f"ig{l}", name=f"ig{l}")
                ig_t[l] = ig
                nc.vector.tensor_mul(ig[:], sig[:, 0:32], tg[:])
                if t > 0:
                    nc.vector.tensor_add(c_t[:], ig[:], fc_t[l][:])
                else:
                    nc.vector.tensor_copy(c_t[:], ig[:])

            def act_tc(l, t):
                c_t = c1 if l == 0 else c2
                tc_ = wk.tile([128, 32], F32, tag=f"tc{l}", name=f"tc{l}")
                tc_t[l] = tc_
                nc.scalar.activation(tc_[:], c_t[:],
                                     mybir.ActivationFunctionType.Tanh,
                                     bias=zb[:])

            def dve_h(l, t):
                sig = sig_t[l]
                if l == 0:
                    hdst = hist[:, (t % SBLK) * 32:(t % SBLK) * 32 + 32]
                else:
                    hdst = h2[:]
                nc.vector.tensor_mul(hdst, sig[:, 64:96], tc_t[l][:])
                if l == 1 and t == t_steps - 1:
                    nc.vector.tensor_mul(h2f[:], sig[:, 64:96], tc_t[l][:])

            # ---- main wave loop: wave w runs l0 step w and l1 step w-8
            for w in range(t_steps + SBLK):
                t0 = w if w < t_steps else None                  # layer-0 step
                t1 = w - SBLK if w >= SBLK else None             # layer-1 step
                s = w % SBLK

                # block-boundary tile switches (prep happened in prior waves)
                if s == 0:
                    if t1 is not None:
                        xp_cur[1] = xp_next1[0]
                        xp_next1[0] = None
                    if t0 is not None:
                        if w == 0:
                            xp_next0[0] = new_tile(0)
                            bias_mms(0, xp_next0[0])
                            xproj_mms(0, 0, range(NCH))
                        xp_cur[0] = xp_next0[0]
                        xp_next0[0] = None

                # recurrent bursts: l1 first (its chain leads the wave)
                if t1 is not None:
                    rec_mms(1, t1)
                if t0 is not None:
                    rec_mms(0, t0)

                # prep work: deprioritized (~2 waves later) so the scheduler
                # fits it into PE idle AFTER both chains' recurrent bursts --
                # without this the scheduler runs prep between the bursts,
                # pushing rec0 and (via the ACT FIFO) the layer-1 chain.
                with tc.high_priority(offset=-110):
                    # layer-1 xproj for ring slots {6,7} of the block starting
                    # now (slot 7 landed at the end of the previous wave):
                    if s == 0 and t1 is not None:
                        xproj1_mms(xp_cur[1], 6, 8)
                    # layer-1 tile for the next block: bias at s=1, then xproj
                    # of ring-slot pairs as soon as each pair of h1 lands
                    k1 = w // SBLK
                    if k1 < NB:
                        if s == 1:
                            xp_next1[0] = new_tile(1)
                            bias_mms(1, xp_next1[0])
                        elif s == 2:
                            xproj1_mms(xp_next1[0], 0, 2)
                        elif s == 4:
                            xproj1_mms(xp_next1[0], 2, 4)
                        elif s == 6:
                            xproj1_mms(xp_next1[0], 4, 6)
                    # layer-0 prep for the next block (x resident; free order)
                    if t0 is not None and t0 // SBLK + 1 < NB:
                        nk = t0 // SBLK + 1
                        if s == 1:
                            xp_next0[0] = new_tile(0)
                            bias_mms(0, xp_next0[0])
                            xproj_mms(0, nk, (0, 1))
                        elif s == 2:
                            xproj_mms(0, nk, (2, 3))
                        elif s == 3:
                            xproj_mms(0, nk, (4, 5))
                        elif s == 4:
                            xproj_mms(0, nk, (6, 7))

                # anti-phased post-matmul stages
                if t1 is not None:
                    act_sig(1, t1)
                    dve_c(1, t1)
                if t0 is not None:
                    act_sig(0, t0)
                if t1 is not None:
                    act_tc(1, t1)
                if t0 is not None:
                    dve_c(0, t0)
                if t1 is not None:
                    dve_h(1, t1)
                if t0 is not None:
                    act_tc(0, t0)
                    dve_h(0, t0)

            # ---- LayerNorm over H on h2f (h2.T layout) -> y [16, 256]
            pt = ps.tile([16, 256], F32, tag="xp0", name="pt")
            nc.tensor.transpose(pt[:, 0:128], h2f[:, 0:16], ident[:])
            nc.tensor.transpose(pt[:, 128:256], h2f[:, 16:32], ident[:])
            hb = wk.tile([16, 256], F32, tag="hb", name="hb")
            nc.vector.tensor_copy(hb[:], pt[:])
            dum = wk.tile([16, 256], F32, tag="dum", name="dum")
            acc = wk.tile([16, 1], F32, tag="acc", name="acc")
            nc.scalar.activation(dum[:], hb[:], mybir.ActivationFunctionType.Copy,
                                 accum_out=acc[:])
            mu = wk.tile([16, 1], F32, tag="mu", name="mu")
            nc.vector.tensor_scalar_mul(mu[:], acc[:], 1.0 / H)
            cen = wk.tile([16, 256], F32, tag="cen", name="cen")
            nc.vector.tensor_scalar_sub(cen[:], hb[:], mu[:])
            acc2 = wk.tile([16, 1], F32, tag="acc2", name="acc2")
            nc.scalar.activation(dum[:], cen[:], mybir.ActivationFunctionType.Square,
                                 bias=zb[0:16, :], accum_out=acc2[:])
            sd = wk.tile([16, 1], F32, tag="sd", name="sd")
            nc.scalar.activation(sd[:], acc2[:], mybir.ActivationFunctionType.Sqrt,
                                 scale=1.0 / H, bias=eps16[:])
            rstd = wk.tile([16, 1], F32, tag="rstd", name="rstd")
            nc.vector.reciprocal(rstd[:], sd[:])
            nrm = wk.tile([16, 256], F32, tag="nrm", name="nrm")
            nc.vector.tensor_scalar_mul(nrm[:], cen[:], rstd[:])
            gam = wk.tile([16, 256], F32, tag="gam", name="gam")
            nc.sync.dma_start(gam[:], gam_d[:])
            bet = wk.tile([16, 256], F32, tag="bet", name="bet")
            nc.sync.dma_start(bet[:], bet_d[:])
            nc.vector.tensor_mul(nrm[:], nrm[:], gam[:])
            out = wk.tile([16, 256], F32, tag="out", name="out")
            nc.vector.tensor_add(out[:], nrm[:], bet[:])
            nc.sync.dma_start(y_d[:], out[:])

    _split_excess_waits(nc)
    return nc


def prep_inputs(x, W_ih, W_hh, b_ih, b_hh, ln_gamma, ln_beta, t_steps=T):
    """host-side shard + transpose + cast. Returns per-core input dicts."""
    bf = ml_dtypes.bfloat16
    # scale the g-gate rows (PERM'd rows 768:1024) by 2: tanh(g)=2*sig(2g)-1
    gscale = np.ones((G4, 1), np.float32)
    gscale[3 * H:] = 2.0
    wih_p = W_ih[:, PERM, :] * gscale
    whh_p = W_hh[:, PERM, :] * gscale
    bias_p = (b_ih + b_hh)[:, PERM] * gscale[:, 0]
    wih = np.ascontiguousarray(np.transpose(wih_p, (0, 2, 1))).reshape(NUM_LAYERS, 2, 128, G4)
    whh = np.ascontiguousarray(np.transpose(whh_p, (0, 2, 1))).reshape(NUM_LAYERS, 2, 128, G4)
    biasm = bias_p.reshape(NUM_LAYERS, 2, 4, 128)
    ind = np.zeros((4, 512), np.float32)
    for k in range(4):
        ind[k, k * 128:(k + 1) * 128] = 1.0
    ident = np.eye(128, dtype=np.float32)
    ins = []
    for cid in range(N_CORES):
        xs = x[cid * BL:(cid + 1) * BL, :t_steps, :]        # [16, t, 256]
        xtp = np.transpose(xs, (2, 1, 0)).reshape(F, t_steps * BL)  # [256, t*16]
        ins.append({
            "xt": np.ascontiguousarray(xtp.reshape(2, 128, t_steps * BL)).astype(bf),
            "wih": wih.astype(bf), "whh": whh.astype(bf),
            "biasm": biasm.astype(bf), "ind": ind.astype(bf),
            "ident": ident,
            "gam": np.broadcast_to(ln_gamma, (BL, H)).astype(np.float32).copy(),
            "bet": np.broadcast_to(ln_beta, (BL, H)).astype(np.float32).copy(),
        })
    return ins


_CACHED = {}


def kernel(x, W_ih, W_hh, b_ih, b_hh, ln_gamma, ln_beta):
    from concourse.bass_utils import run_bass_kernel_spmd
    x = np.asarray(x, dtype=np.float32)
    ins = prep_inputs(np.asarray(x), np.asarray(W_ih), np.asarray(W_hh),
                      np.asarray(b_ih), np.asarray(b_hh),
                      np.asarray(ln_gamma), np.asarray(ln_beta))
    if "nc" not in _CACHED:
        _CACHED["nc"] = build(T)
    res = run_bass_kernel_spmd(_CACHED["nc"], ins, core_ids=list(range(N_CORES)))
    return np.concatenate([res.results[c]["y"] for c in range(N_CORES)], axis=0)


# revision 13
# speedup vs baseline: 1.3075x; 1.0096x over previous
"""2-layer LSTM (B=128, T=1024, H=256) + last-step LayerNorm on 8 trn2 cores.

Data-parallel over batch (16 rows/core). Per core, everything is kept in a
transposed layout (hidden/gate dims on partitions, batch on the free axis):

  - gates.T for a block of 8 timesteps live in one PSUM tile [128, 1024] per
    layer (col = 128*chunk + 16*step_in_block + b). Each block's tile is
    initialized by two N=512 "indicator" matmuls that write the bias to the
    whole bank (start=True clears has_written), then the x-projection
    (Wih.T chunks stationary, x.T streaming) and per-step recurrent terms
    (Whh.T chunks stationary fp8, h.T streaming) accumulate on top.
  - the per-step chain is kept short: ONE sigmoid per layer covers all four
    gates (g rows are pre-scaled by 2 on the host so tanh(g) = 2*sig(2g)-1;
    the affine is a single fused tensor_scalar on DVE), then c/h updates are
    short [128, 32] vector ops, then tanh(c) and the h write (bf16, where
    the next matmul streams it from).
  - layer 2 runs 8 steps behind layer 1; per-wave issue order anti-phases
    the two chains across the PE/ACT/DVE FIFOs so one chain's matmul burst
    overlaps the other's activation/vector stages.
  - all operands bf16 (fp8 Whh was tried: no speedup -- the recurrent burst
    is MM-issue-floor-bound at ~27ns/tile either way -- so bf16 keeps the
    better numerics), accumulation fp32, c stays fp32.

Final step: PE transpose of h2 back to [16, 256], LayerNorm, DMA out.
"""
import sys

sys.path.insert(0, "/opt/trn_rl_repo")

import numpy as np
import ml_dtypes

import concourse.bass as bass
import concourse.mybir as mybir
import concourse.tile as tile

NUM_LAYERS = 2
H = 256
F = 256
B, T = 128, 1024
LN_EPS = 1e-5
N_CORES = 8
BL = B // N_CORES          # batch rows per core = 16
G4 = 4 * H                 # 1024 gate dims
NCH = G4 // 128            # 8 chunks of gate dims
SBLK = 8                   # timesteps per x-projection block
BF16 = mybir.dt.bfloat16
F32 = mybir.dt.float32

# gate reorder: torch (i,f,g,o) -> (i,f,o,g)
PERM = np.concatenate([np.arange(0, 2 * H), np.arange(3 * H, 4 * H),
                       np.arange(2 * H, 3 * H)])


def _split_excess_waits(nc):
    """walrus in this container rejects instructions with >1 sem wait
    (CoreV3 setupSyncWait). Hoist excess waits onto NoOps just before."""
    for fn in nc.m.functions:
        for blk in fn.blocks:
            insts = list(blk.instructions)
            out, n_new = [], 0
            for inst in insts:
                si = inst.sync_info
                waits = list(si.on_wait) if si is not None else []
                if len(waits) > 1:
                    head, rest = waits[:-1], waits[-1:]
                    for wt in head:
                        nop = mybir.InstNoOp(
                            name=f"{inst.name}-ws{n_new}",
                            engine=inst.engine,
                            ins=[], outs=[],
                            sync_info=mybir.SyncInfo(on_wait=[wt], on_update=[]),
                        )
                        n_new += 1
                        out.append(nop)
                    inst.sync_info = mybir.SyncInfo(
                        on_wait=rest, on_update=list(si.on_update))
                out.append(inst)
            if n_new:
                try:
                    blk.instructions = out
                except Exception:
                    blk.set_instructions(out)


def build(t_steps=T):
    nc = bass.Bass()
    TB = t_steps * BL
    xt_d = nc.dram_tensor("xt", [2, 128, TB], BF16, kind="ExternalInput")
    wih_d = nc.dram_tensor("wih", [NUM_LAYERS, 2, 128, G4], BF16, kind="ExternalInput")
    whh_d = nc.dram_tensor("whh", [NUM_LAYERS, 2, 128, G4], BF16, kind="ExternalInput")
    biasm_d = nc.dram_tensor("biasm", [NUM_LAYERS, 2, 4, 128], BF16, kind="ExternalInput")
    ind_d = nc.dram_tensor("ind", [4, 512], BF16, kind="ExternalInput")
    ident_d = nc.dram_tensor("ident", [128, 128], F32, kind="ExternalInput")
    gam_d = nc.dram_tensor("gam", [BL, H], F32, kind="ExternalInput")
    bet_d = nc.dram_tensor("bet", [BL, H], F32, kind="ExternalInput")
    y_d = nc.dram_tensor("y", [BL, H], F32, kind="ExternalOutput")

    NB = t_steps // SBLK
    with tile.TileContext(nc) as tc:
        with (
            tc.tile_pool(name="wts", bufs=1) as wts,
            tc.tile_pool(name="state", bufs=1) as st,
            tc.tile_pool(name="work", bufs=4) as wk,
            tc.tile_pool(name="psum", bufs=2, space="PSUM") as ps,
        ):
            # resident tensors (partition dim first on every SBUF tile)
            xt = [wts.tile([128, TB], BF16, tag=f"xt{kw}", name=f"xt{kw}") for kw in (0, 1)]
            for kw in (0, 1):
                nc.sync.dma_start(xt[kw][:], xt_d[kw])
            wih = [[wts.tile([128, G4], BF16, tag=f"wih{l}{kw}", name=f"wih{l}{kw}") for kw in (0, 1)]
                   for l in range(NUM_LAYERS)]
            whh = [[wts.tile([128, G4], BF16, tag=f"whh{l}{kw}", name=f"whh{l}{kw}") for kw in (0, 1)]
                   for l in range(NUM_LAYERS)]
            biasm = [[wts.tile([4, 128], BF16, tag=f"bm{l}{b}", name=f"bm{l}{b}") for b in (0, 1)]
                     for l in range(NUM_LAYERS)]
            for l in range(NUM_LAYERS):
                for kw in (0, 1):
                    nc.sync.dma_start(wih[l][kw][:], wih_d[l, kw])
                    nc.sync.dma_start(whh[l][kw][:], whh_d[l, kw])
                for b in (0, 1):
                    nc.sync.dma_start(biasm[l][b][:], biasm_d[l, b])
            ind = wts.tile([4, 512], BF16, tag="ind", name="ind")
            nc.sync.dma_start(ind[:], ind_d[:])
            ident = wts.tile([128, 128], F32, tag="ident", name="ident")
            nc.sync.dma_start(ident[:], ident_d[:])
            zb = wts.tile([128, 1], F32, tag="zb", name="zb")
            nc.vector.memset(zb[:], 0.0)
            eps16 = wts.tile([16, 1], F32, tag="eps16", name="eps16")
            nc.vector.memset(eps16[:], LN_EPS)

            # persistent state
            hist = st.tile([128, SBLK * 32], BF16, tag="hist", name="hist")   # layer-1 h ring
            h2 = st.tile([128, 32], BF16, tag="h2", name="h2")
            c1 = st.tile([128, 32], F32, tag="c1", name="c1")
            c2 = st.tile([128, 32], F32, tag="c2", name="c2")
            h2f = st.tile([128, 32], F32, tag="h2f", name="h2f")

            xp_cur = [None, None]   # current psum block tile per layer
            xp_next0 = [None]       # layer-0 tile being prepped for next block
            xp_next1 = [None]       # layer-1 tile being prepped for next block

            def new_tile(l):
                return ps.tile([128, NCH * 128], F32, tag=f"xp{l}", name=f"xp{l}")

            def bias_mms(l, xp):
                for b in (0, 1):
                    nc.tensor.matmul(
                        xp[:, b * 512:(b + 1) * 512], biasm[l][b][:], ind[:],
                        start=True, stop=False, skip_group_check=True)

            def xproj_mms(l, k, chunks):
                """x-projection matmuls (all slots) for block k of layer 0."""
                xp = xp_next0[0]
                rhs = [xt[kw][:, k * SBLK * BL:(k + 1) * SBLK * BL] for kw in (0, 1)]
                for ch in chunks:
                    o = xp[:, ch * 128:(ch + 1) * 128]
                    for kw in (0, 1):
                        nc.tensor.matmul(
                            o, wih[l][kw][:, ch * 128:(ch + 1) * 128], rhs[kw],
                            start=False, stop=False, skip_group_check=True)

            def xproj1_mms(xp, s0, s1, chunks=range(NCH)):
                """layer-1 x-projection for ring slots [s0, s1)."""
                hv = hist[:].rearrange("p (s w) -> p s w", s=SBLK)
                rhs = [hv[:, s0:s1, 0:BL], hv[:, s0:s1, BL:2 * BL]]
                for ch in chunks:
                    o = xp[:, ch * 128 + s0 * 16: ch * 128 + s1 * 16]
                    for kw in (0, 1):
                        nc.tensor.matmul(
                            o, wih[1][kw][:, ch * 128:(ch + 1) * 128], rhs[kw],
                            start=False, stop=False, skip_group_check=True)

            def rec_mms(l, t):
                """recurrent matmuls for one step (all 8 chunks x 2 kw)."""
                if t == 0:
                    return
                s = t % SBLK
                xp = xp_cur[l]
                if l == 0:
                    hsrc = hist[:, ((t - 1) % SBLK) * 32:((t - 1) % SBLK) * 32 + 32]
                else:
                    hsrc = h2[:]
                for ch in range(NCH):
                    o = xp[:, ch * 128 + 16 * s: ch * 128 + 16 * s + 16]
                    for kw in (0, 1):
                        nc.tensor.matmul(
                            o, whh[l][kw][:, ch * 128:(ch + 1) * 128],
                            hsrc[:, 16 * kw:16 * kw + 16],
                            start=False, stop=(kw == 1), skip_group_check=True)

            sig_t = [None, None]
            tg_t = [None, None]
            ig_t = [None, None]
            fc_t = [None, None]
            tc_t = [None, None]

            def act_sig(l, t):
                """one sigmoid over all 4 gates (g pre-scaled x2 on host)."""
                s = t % SBLK
                xpv = xp_cur[l][:].rearrange("p (c s w) -> p c s w", c=NCH, s=SBLK)
                sig = wk.tile([128, 128], F32, tag=f"sig{l}", name=f"sig{l}")
                sig_t[l] = sig
                nc.scalar.activation(sig[:].rearrange("p (c w) -> p c w", c=NCH),
                                     xpv[:, 0:NCH, s, :],
                                     mybir.ActivationFunctionType.Sigmoid,
                                     bias=zb[:])

            def dve_c(l, t):
                """tanh(g) affine + c update."""
                sig = sig_t[l]
                c_t = c1 if l == 0 else c2
                tg = wk.tile([128, 32], F32, tag=f"tg{l}", name=f"tg{l}")
                tg_t[l] = tg
                nc.vector.tensor_scalar(tg[:], sig[:, 96:128], 2.0, 1.0,
                                        mybir.AluOpType.mult,
                                        mybir.AluOpType.subtract)
                if t > 0:
                    fc = wk.tile([128, 32], F32, tag=f"fc{l}", name=f"fc{l}")
                    fc_t[l] = fc
                    nc.vector.tensor_mul(fc[:], sig[:, 32:64], c_t[:])
                ig = wk.tile([128, 32], F32, tag=f"ig{l}", name=f"ig{l}")
                ig_t[l] = ig
                nc.vector.tensor_mul(ig[:], sig[:, 0:32], tg[:])
                if t > 0:
                    nc.vector.tensor_add(c_t[:], ig[:], fc_t[l][:])
                else:
                    nc.vector.tensor_copy(c_t[:], ig[:])

            def act_tc(l, t):
                c_t = c1 if l == 0 else c2
                tc_ = wk.tile([128, 32], F32, tag=f"tc{l}", name=f"tc{l}")
                tc_t[l] = tc_
                nc.scalar.activation(tc_[:], c_t[:],
                                     mybir.ActivationFunctionType.Tanh,
                                     bias=zb[:])

            def dve_h(l, t):
                sig = sig_t[l]
                if l == 0:
                    hdst = hist[:, (t % SBLK) * 32:(t % SBLK) * 32 + 32]
                else:
                    hdst = h2[:]
                nc.vector.tensor_mul(hdst, sig[:, 64:96], tc_t[l][:])
                if l == 1 and t == t_steps - 1:
                    nc.vector.tensor_mul(h2f[:], sig[:, 64:96], tc_t[l][:])

            # ---- main wave loop: wave w runs l0 step w and l1 step w-8
            for w in range(t_steps + SBLK):
                t0 = w if w < t_steps else None                  # layer-0 step
                t1 = w - SBLK if w >= SBLK else None             # layer-1 step
                s = w % SBLK

                # block-boundary tile switches (prep happened in prior waves)
                if s == 0:
                    if t1 is not None:
                        xp_cur[1] = xp_next1[0]
                        xp_next1[0] = None
                    if t0 is not None:
                        if w == 0:
                            xp_next0[0] = new_tile(0)
                            bias_mms(0, xp_next0[0])
                            xproj_mms(0, 0, range(NCH))
                        xp_cur[0] = xp_next0[0]
                        xp_next0[0] = None

                # recurrent bursts: l1 first (its chain leads the wave)
                if t1 is not None:
                    rec_mms(1, t1)
                if t0 is not None:
                    rec_mms(0, t0)

                # prep work, in small (<=8 MM) pieces: the greedy scheduler
                # slots each piece into whatever PE gap exists, and a small
                # piece's tail can only push a stalled recurrent burst (and,
                # via the ACT FIFO, the chains) by ~0.2us instead of ~0.5us.
                # layer-1 xproj of ring slots {6,7} for the block started at
                # the last boundary (slot 7 landed at the end of wave s=7):
                if t1 is not None:
                    if s == 0:
                        xproj1_mms(xp_cur[1], 6, 8, range(0, 4))
                    elif s == 1:
                        xproj1_mms(xp_cur[1], 6, 8, range(4, NCH))
                # layer-1 tile for the next block: bias at s=1, then xproj of
                # each ring-slot pair in two chunk-halves as the h1s land
                k1 = w // SBLK
                if k1 < NB:
                    if s == 1:
                        xp_next1[0] = new_tile(1)
                        bias_mms(1, xp_next1[0])
                    elif s == 2:
                        xproj1_mms(xp_next1[0], 0, 2, range(0, 4))
                    elif s == 3:
                        xproj1_mms(xp_next1[0], 0, 2, range(4, NCH))
                    elif s == 4:
                        xproj1_mms(xp_next1[0], 2, 4, range(0, 4))
                    elif s == 5:
                        xproj1_mms(xp_next1[0], 2, 4, range(4, NCH))
                    elif s == 6:
                        xproj1_mms(xp_next1[0], 4, 6, range(0, 4))
                    elif s == 7:
                        xproj1_mms(xp_next1[0], 4, 6, range(4, NCH))
                # layer-0 prep for the next block (x resident; free order)
                if t0 is not None and t0 // SBLK + 1 < NB:
                    nk = t0 // SBLK + 1
                    if s == 1:
                        xp_next0[0] = new_tile(0)
                        bias_mms(0, xp_next0[0])
                    elif s == 2:
                        xproj_mms(0, nk, (0, 1))
                    elif s == 3:
                        xproj_mms(0, nk, (2, 3))
                    elif s == 4:
                        xproj_mms(0, nk, (4, 5))
                    elif s == 5:
                        xproj_mms(0, nk, (6, 7))

                # anti-phased post-matmul stages
                if t1 is not None:
                    act_sig(1, t1)
                    dve_c(1, t1)
                if t0 is not None:
                    act_sig(0, t0)
                if t1 is not None:
                    act_tc(1, t1)
                if t0 is not None:
                    dve_c(0, t0)
                if t1 is not None:
                    dve_h(1, t1)
                if t0 is not None:
                    act_tc(0, t0)
                    dve_h(0, t0)

            # ---- LayerNorm over H on h2f (h2.T layout) -> y [16, 256]
            pt = ps.tile([16, 256], F32, tag="xp0", name="pt")
            nc.tensor.transpose(pt[:, 0:128], h2f[:, 0:16], ident[:])
            nc.tensor.transpose(pt[:, 128:256], h2f[:, 16:32], ident[:])
            hb = wk.tile([16, 256], F32, tag="hb", name="hb")
            nc.vector.tensor_copy(hb[:], pt[:])
            dum = wk.tile([16, 256], F32, tag="dum", name="dum")
            acc = wk.tile([16, 1], F32, tag="acc", name="acc")
            nc.scalar.activation(dum[:], hb[:], mybir.ActivationFunctionType.Copy,
                                 accum_out=acc[:])
            mu = wk.tile([16, 1], F32, tag="mu", name="mu")
            nc.vector.tensor_scalar_mul(mu[:], acc[:], 1.0 / H)
            cen = wk.tile([16, 256], F32, tag="cen", name="cen")
            nc.vector.tensor_scalar_sub(cen[:], hb[:], mu[:])
            acc2 = wk.tile([16, 1], F32, tag="acc2", name="acc2")
            nc.scalar.activation(dum[:], cen[:], mybir.ActivationFunctionType.Square,
                                 bias=zb[0:16, :], accum_out=acc2[:])
            sd = wk.tile([16, 1], F32, tag="sd", name="sd")
            nc.scalar.activation(sd[:], acc2[:], mybir.ActivationFunctionType.Sqrt,
                                 scale=1.0 / H, bias=eps16[:])
            rstd = wk.tile([16, 1], F32, tag="rstd", name="rstd")
            nc.vector.reciprocal(rstd[:], sd[:])
            nrm = wk.tile([16, 256], F32, tag="nrm", name="nrm")
            nc.vector.tensor_scalar_mul(nrm[:], cen[:], rstd[:])
            gam = wk.tile([16, 256], F32, tag="gam", name="gam")
            nc.sync.dma_start(gam[:], gam_d[:])
            bet = wk.tile([16, 256], F32, tag="bet", name="bet")
            nc.sync.dma_start(bet[:], bet_d[:])
            nc.vector.tensor_mul(nrm[:], nrm[:], gam[:])
            out = wk.tile([16, 256], F32, tag="out", name="out")
            nc.vector.tensor_add(out[:], nrm[:], bet[:])
            nc.sync.dma_start(y_d[:], out[:])

    _split_excess_waits(nc)
    return nc


def prep_inputs(x, W_ih, W_hh, b_ih, b_hh, ln_gamma, ln_beta, t_steps=T):
    """host-side shard + transpose + cast. Returns per-core input dicts."""
    bf = ml_dtypes.bfloat16
    # scale the g-gate rows (PERM'd rows 768:1024) by 2: tanh(g)=2*sig(2g)-1
    gscale = np.ones((G4, 1), np.float32)
    gscale[3 * H:] = 2.0
    wih_p = W_ih[:, PERM, :] * gscale
    whh_p = W_hh[:, PERM, :] * gscale
    bias_p = (b_ih + b_hh)[:, PERM] * gscale[:, 0]
    wih = np.ascontiguousarray(np.transpose(wih_p, (0, 2, 1))).reshape(NUM_LAYERS, 2, 128, G4)
    whh = np.ascontiguousarray(np.transpose(whh_p, (0, 2, 1))).reshape(NUM_LAYERS, 2, 128, G4)
    biasm = bias_p.reshape(NUM_LAYERS, 2, 4, 128)
    ind = np.zeros((4, 512), np.float32)
    for k in range(4):
        ind[k, k * 128:(k + 1) * 128] = 1.0
    ident = np.eye(128, dtype=np.float32)
    ins = []
    for cid in range(N_CORES):
        xs = x[cid * BL:(cid + 1) * BL, :t_steps, :]        # [16, t, 256]
        xtp = np.transpose(xs, (2, 1, 0)).reshape(F, t_steps * BL)  # [256, t*16]
        ins.append({
            "xt": np.ascontiguousarray(xtp.reshape(2, 128, t_steps * BL)).astype(bf),
            "wih": wih.astype(bf), "whh": whh.astype(bf),
            "biasm": biasm.astype(bf), "ind": ind.astype(bf),
            "ident": ident,
            "gam": np.broadcast_to(ln_gamma, (BL, H)).astype(np.float32).copy(),
            "bet": np.broadcast_to(ln_beta, (BL, H)).astype(np.float32).copy(),
        })
    return ins


_CACHED = {}


def kernel(x, W_ih, W_hh, b_ih, b_hh, ln_gamma, ln_beta):
    from concourse.bass_utils import run_bass_kernel_spmd
    x = np.asarray(x, dtype=np.float32)
    ins = prep_inputs(np.asarray(x), np.asarray(W_ih), np.asarray(W_hh),
                      np.asarray(b_ih), np.asarray(b_hh),
                      np.asarray(ln_gamma), np.asarray(ln_beta))
    if "nc" not in _CACHED:
        _CACHED["nc"] = build(T)
    res = run_bass_kernel_spmd(_CACHED["nc"], ins, core_ids=list(range(N_CORES)))
    return np.concatenate([res.results[c]["y"] for c in range(N_CORES)], axis=0)


# revision 15
# speedup vs baseline: 1.3328x; 1.0194x over previous
"""2-layer LSTM (B=128, T=1024, H=256) + last-step LayerNorm on 8 trn2 cores.

Data-parallel over batch (16 rows/core). Per core, everything is kept in a
transposed layout (hidden/gate dims on partitions, batch on the free axis):

  - gates.T for a block of 8 timesteps live in one PSUM tile [128, 1024] per
    layer (col = 128*chunk + 16*step_in_block + b). Each block's tile is
    initialized by two N=512 "indicator" matmuls that write the bias to the
    whole bank (start=True clears has_written), then the x-projection
    (Wih.T chunks stationary, x.T streaming) and per-step recurrent terms
    (Whh.T chunks stationary fp8, h.T streaming) accumulate on top.
  - the per-step chain is kept short: ONE sigmoid per layer covers all four
    gates (g rows are pre-scaled by 2 on the host so tanh(g) = 2*sig(2g)-1;
    the affine is a single fused tensor_scalar on DVE), then c/h updates are
    short [128, 32] vector ops, then tanh(c) and the h write (bf16, where
    the next matmul streams it from).
  - layer 2 runs 8 steps behind layer 1; per-wave issue order anti-phases
    the two chains across the PE/ACT/DVE FIFOs so one chain's matmul burst
    overlaps the other's activation/vector stages.
  - all operands bf16 (fp8 Whh was tried: no speedup -- the recurrent burst
    is MM-issue-floor-bound at ~27ns/tile either way -- so bf16 keeps the
    better numerics), accumulation fp32, c stays fp32.

Final step: PE transpose of h2 back to [16, 256], LayerNorm, DMA out.
"""
import sys

sys.path.insert(0, "/opt/trn_rl_repo")

import numpy as np
import ml_dtypes

import concourse.bass as bass
import concourse.mybir as mybir
import concourse.tile as tile

NUM_LAYERS = 2
H = 256
F = 256
B, T = 128, 1024
LN_EPS = 1e-5
N_CORES = 8
BL = B // N_CORES          # batch rows per core = 16
G4 = 4 * H                 # 1024 gate dims
NCH = G4 // 128            # 8 chunks of gate dims
SBLK = 8                   # timesteps per x-projection block
BF16 = mybir.dt.bfloat16
F32 = mybir.dt.float32

# gate reorder: torch (i,f,g,o) -> (i,f,o,g)
PERM = np.concatenate([np.arange(0, 2 * H), np.arange(3 * H, 4 * H),
                       np.arange(2 * H, 3 * H)])


def _split_excess_waits(nc):
    """walrus in this container rejects instructions with >1 sem wait
    (CoreV3 setupSyncWait). Hoist excess waits onto NoOps just before."""
    for fn in nc.m.functions:
        for blk in fn.blocks:
            insts = list(blk.instructions)
            out, n_new = [], 0
            for inst in insts:
                si = inst.sync_info
                waits = list(si.on_wait) if si is not None else []
                if len(waits) > 1:
                    head, rest = waits[:-1], waits[-1:]
                    for wt in head:
                        nop = mybir.InstNoOp(
                            name=f"{inst.name}-ws{n_new}",
                            engine=inst.engine,
                            ins=[], outs=[],
                            sync_info=mybir.SyncInfo(on_wait=[wt], on_update=[]),
                        )
                        n_new += 1
                        out.append(nop)
                    inst.sync_info = mybir.SyncInfo(
                        on_wait=rest, on_update=list(si.on_update))
                out.append(inst)
            if n_new:
                try:
                    blk.instructions = out
                except Exception:
                    blk.set_instructions(out)


def build(t_steps=T):
    nc = bass.Bass()
    TB = t_steps * BL
    xt_d = nc.dram_tensor("xt", [2, 128, TB], BF16, kind="ExternalInput")
    wih_d = nc.dram_tensor("wih", [NUM_LAYERS, 2, 128, G4], BF16, kind="ExternalInput")
    whh_d = nc.dram_tensor("whh", [NUM_LAYERS, 2, 128, G4], BF16, kind="ExternalInput")
    biasm_d = nc.dram_tensor("biasm", [NUM_LAYERS, 2, 4, 128], BF16, kind="ExternalInput")
    ind_d = nc.dram_tensor("ind", [4, 512], BF16, kind="ExternalInput")
    ident_d = nc.dram_tensor("ident", [128, 128], F32, kind="ExternalInput")
    gam_d = nc.dram_tensor("gam", [BL, H], F32, kind="ExternalInput")
    bet_d = nc.dram_tensor("bet", [BL, H], F32, kind="ExternalInput")
    y_d = nc.dram_tensor("y", [BL, H], F32, kind="ExternalOutput")

    NB = t_steps // SBLK
    with tile.TileContext(nc) as tc:
        with (
            tc.tile_pool(name="wts", bufs=1) as wts,
            tc.tile_pool(name="state", bufs=1) as st,
            tc.tile_pool(name="work", bufs=4) as wk,
            tc.tile_pool(name="psum", bufs=2, space="PSUM") as ps,
        ):
            # resident tensors (partition dim first on every SBUF tile)
            xt = [wts.tile([128, TB], BF16, tag=f"xt{kw}", name=f"xt{kw}") for kw in (0, 1)]
            for kw in (0, 1):
                nc.sync.dma_start(xt[kw][:], xt_d[kw])
            wih = [[wts.tile([128, G4], BF16, tag=f"wih{l}{kw}", name=f"wih{l}{kw}") for kw in (0, 1)]
                   for l in range(NUM_LAYERS)]
            whh = [[wts.tile([128, G4], BF16, tag=f"whh{l}{kw}", name=f"whh{l}{kw}") for kw in (0, 1)]
                   for l in range(NUM_LAYERS)]
            biasm = [[wts.tile([4, 128], BF16, tag=f"bm{l}{b}", name=f"bm{l}{b}") for b in (0, 1)]
                     for l in range(NUM_LAYERS)]
            for l in range(NUM_LAYERS):
                for kw in (0, 1):
                    nc.sync.dma_start(wih[l][kw][:], wih_d[l, kw])
                    nc.sync.dma_start(whh[l][kw][:], whh_d[l, kw])
                for b in (0, 1):
                    nc.sync.dma_start(biasm[l][b][:], biasm_d[l, b])
            ind = wts.tile([4, 512], BF16, tag="ind", name="ind")
            nc.sync.dma_start(ind[:], ind_d[:])
            ident = wts.tile([128, 128], F32, tag="ident", name="ident")
            nc.sync.dma_start(ident[:], ident_d[:])
            zb = wts.tile([128, 1], F32, tag="zb", name="zb")
            nc.vector.memset(zb[:], 0.0)
            eps16 = wts.tile([16, 1], F32, tag="eps16", name="eps16")
            nc.vector.memset(eps16[:], LN_EPS)

            # persistent state
            hist = st.tile([128, SBLK * 32], BF16, tag="hist", name="hist")   # layer-1 h ring
            h2 = st.tile([128, 32], BF16, tag="h2", name="h2")
            c1 = st.tile([128, 32], F32, tag="c1", name="c1")
            c2 = st.tile([128, 32], F32, tag="c2", name="c2")
            h2f = st.tile([128, 32], F32, tag="h2f", name="h2f")

            xp_cur = [None, None]   # current psum block tile per layer
            xp_next0 = [None]       # layer-0 tile being prepped for next block
            xp_next1 = [None]       # layer-1 tile being prepped for next block

            def new_tile(l):
                return ps.tile([128, NCH * 128], F32, tag=f"xp{l}", name=f"xp{l}")

            def bias_mms(l, xp, banks=(0, 1)):
                for b in banks:
                    nc.tensor.matmul(
                        xp[:, b * 512:(b + 1) * 512], biasm[l][b][:], ind[:],
                        start=True, stop=False, skip_group_check=True)

            def xproj_mms(l, k, chunks):
                """x-projection matmuls (all slots) for block k of layer 0."""
                xp = xp_next0[0]
                rhs = [xt[kw][:, k * SBLK * BL:(k + 1) * SBLK * BL] for kw in (0, 1)]
                for ch in chunks:
                    o = xp[:, ch * 128:(ch + 1) * 128]
                    for kw in (0, 1):
                        nc.tensor.matmul(
                            o, wih[l][kw][:, ch * 128:(ch + 1) * 128], rhs[kw],
                            start=False, stop=False, skip_group_check=True)

            def xproj1_mms(xp, s0, s1, chunks=range(NCH)):
                """layer-1 x-projection for ring slots [s0, s1)."""
                hv = hist[:].rearrange("p (s w) -> p s w", s=SBLK)
                rhs = [hv[:, s0:s1, 0:BL], hv[:, s0:s1, BL:2 * BL]]
                for ch in chunks:
                    o = xp[:, ch * 128 + s0 * 16: ch * 128 + s1 * 16]
                    for kw in (0, 1):
                        nc.tensor.matmul(
                            o, wih[1][kw][:, ch * 128:(ch + 1) * 128], rhs[kw],
                            start=False, stop=False, skip_group_check=True)

            def rec_mms(l, t):
                """recurrent matmuls for one step (all 8 chunks x 2 kw)."""
                if t == 0:
                    return
                s = t % SBLK
                xp = xp_cur[l]
                if l == 0:
                    hsrc = hist[:, ((t - 1) % SBLK) * 32:((t - 1) % SBLK) * 32 + 32]
                else:
                    hsrc = h2[:]
                for ch in range(NCH):
                    o = xp[:, ch * 128 + 16 * s: ch * 128 + 16 * s + 16]
                    for kw in (0, 1):
                        nc.tensor.matmul(
                            o, whh[l][kw][:, ch * 128:(ch + 1) * 128],
                            hsrc[:, 16 * kw:16 * kw + 16],
                            start=False, stop=(kw == 1), skip_group_check=True)

            sig_t = [None, None]
            tg_t = [None, None]
            ig_t = [None, None]
            fc_t = [None, None]
            tc_t = [None, None]

            def act_sig(l, t):
                """one sigmoid over all 4 gates (g pre-scaled x2 on host)."""
                s = t % SBLK
                xpv = xp_cur[l][:].rearrange("p (c s w) -> p c s w", c=NCH, s=SBLK)
                sig = wk.tile([128, 128], F32, tag=f"sig{l}", name=f"sig{l}")
                sig_t[l] = sig
                nc.scalar.activation(sig[:].rearrange("p (c w) -> p c w", c=NCH),
                                     xpv[:, 0:NCH, s, :],
                                     mybir.ActivationFunctionType.Sigmoid,
                                     bias=zb[:])

            def dve_c(l, t):
                """tanh(g) affine + c update."""
                sig = sig_t[l]
                c_t = c1 if l == 0 else c2
                tg = wk.tile([128, 32], F32, tag=f"tg{l}", name=f"tg{l}")
                tg_t[l] = tg
                nc.vector.tensor_scalar(tg[:], sig[:, 96:128], 2.0, 1.0,
                                        mybir.AluOpType.mult,
                                        mybir.AluOpType.subtract)
                if t > 0:
                    fc = wk.tile([128, 32], F32, tag=f"fc{l}", name=f"fc{l}")
                    fc_t[l] = fc
                    nc.vector.tensor_mul(fc[:], sig[:, 32:64], c_t[:])
                ig = wk.tile([128, 32], F32, tag=f"ig{l}", name=f"ig{l}")
                ig_t[l] = ig
                nc.vector.tensor_mul(ig[:], sig[:, 0:32], tg[:])
                if t > 0:
                    nc.vector.tensor_add(c_t[:], ig[:], fc_t[l][:])
                else:
                    nc.vector.tensor_copy(c_t[:], ig[:])

            def act_tc(l, t):
                c_t = c1 if l == 0 else c2
                tc_ = wk.tile([128, 32], F32, tag=f"tc{l}", name=f"tc{l}")
                tc_t[l] = tc_
                nc.scalar.activation(tc_[:], c_t[:],
                                     mybir.ActivationFunctionType.Tanh,
                                     bias=zb[:])

            def dve_h(l, t):
                sig = sig_t[l]
                if l == 0:
                    hdst = hist[:, (t % SBLK) * 32:(t % SBLK) * 32 + 32]
                else:
                    hdst = h2[:]
                nc.vector.tensor_mul(hdst, sig[:, 64:96], tc_t[l][:])
                if l == 1 and t == t_steps - 1:
                    nc.vector.tensor_mul(h2f[:], sig[:, 64:96], tc_t[l][:])

            # ---- main wave loop: wave w runs l0 step w and l1 step w-8
            for w in range(t_steps + SBLK):
                t0 = w if w < t_steps else None                  # layer-0 step
                t1 = w - SBLK if w >= SBLK else None             # layer-1 step
                s = w % SBLK

                # block-boundary tile switches (prep happened in prior waves)
                if s == 0:
                    if t1 is not None:
                        xp_cur[1] = xp_next1[0]
                        xp_next1[0] = None
                    if t0 is not None:
                        if w == 0:
                            xp_next0[0] = new_tile(0)
                            bias_mms(0, xp_next0[0])
                            xproj_mms(0, 0, range(NCH))
                        xp_cur[0] = xp_next0[0]
                        xp_next0[0] = None

                # recurrent bursts: l1 first (its chain leads the wave)
                if t1 is not None:
                    rec_mms(1, t1)
                if t0 is not None:
                    rec_mms(0, t0)

                # prep work, evened out: every wave carries ~2 small (4-MM)
                # layer-1 xproj pieces with >=1 wave of data slack, and at
                # most one fat (N=512, ~600ns) bias matmul, so no wave's PE
                # prep tail can push a stalled recurrent burst far.
                # layer-1 pieces for ring-slot pairs of the CURRENT tile
                # ({4,5} and {6,7} complete after the block switch):
                if t1 is not None:
                    if s == 0:
                        xproj1_mms(xp_cur[1], 4, 6, range(0, 4))
                    elif s == 1:
                        xproj1_mms(xp_cur[1], 4, 6, range(4, NCH))
                    elif s == 2:
                        xproj1_mms(xp_cur[1], 6, 8, range(0, 4))
                    elif s == 3:
                        xproj1_mms(xp_cur[1], 6, 8, range(4, NCH))
                # layer-1 tile for the next block: bias banks at s=3,4, then
                # slot-pair xproj pieces one wave after their h1 data lands
                k1 = w // SBLK
                if k1 < NB:
                    if s == 3:
                        xp_next1[0] = new_tile(1)
                        bias_mms(1, xp_next1[0], (0,))
                    elif s == 4:
                        bias_mms(1, xp_next1[0], (1,))
                        xproj1_mms(xp_next1[0], 0, 2, range(0, 4))
                    elif s == 5:
                        xproj1_mms(xp_next1[0], 0, 2, range(4, NCH))
                    elif s == 6:
                        xproj1_mms(xp_next1[0], 2, 4, range(0, 4))
                    elif s == 7:
                        xproj1_mms(xp_next1[0], 2, 4, range(4, NCH))
                # layer-0 prep for the next block (x resident; free order)
                if t0 is not None and t0 // SBLK + 1 < NB:
                    nk = t0 // SBLK + 1
                    if s == 1:
                        xp_next0[0] = new_tile(0)
                        bias_mms(0, xp_next0[0], (0,))
                    elif s == 2:
                        bias_mms(0, xp_next0[0], (1,))
                        xproj_mms(0, nk, (0, 1))
                    elif s == 3:
                        xproj_mms(0, nk, (2, 3))
                    elif s == 4:
                        xproj_mms(0, nk, (4, 5))
                    elif s == 5:
                        xproj_mms(0, nk, (6, 7))

                # anti-phased post-matmul stages
                if t1 is not None:
                    act_sig(1, t1)
                    dve_c(1, t1)
                if t0 is not None:
                    act_sig(0, t0)
                if t1 is not None:
                    act_tc(1, t1)
                if t0 is not None:
                    dve_c(0, t0)
                if t1 is not None:
                    dve_h(1, t1)
                if t0 is not None:
                    act_tc(0, t0)
                    dve_h(0, t0)

            # ---- LayerNorm over H on h2f (h2.T layout) -> y [16, 256]
            pt = ps.tile([16, 256], F32, tag="xp0", name="pt")
            nc.tensor.transpose(pt[:, 0:128], h2f[:, 0:16], ident[:])
            nc.tensor.transpose(pt[:, 128:256], h2f[:, 16:32], ident[:])
            hb = wk.tile([16, 256], F32, tag="hb", name="hb")
            nc.vector.tensor_copy(hb[:], pt[:])
            dum = wk.tile([16, 256], F32, tag="dum", name="dum")
            acc = wk.tile([16, 1], F32, tag="acc", name="acc")
            nc.scalar.activation(dum[:], hb[:], mybir.ActivationFunctionType.Copy,
                                 accum_out=acc[:])
            mu = wk.tile([16, 1], F32, tag="mu", name="mu")
            nc.vector.tensor_scalar_mul(mu[:], acc[:], 1.0 / H)
            cen = wk.tile([16, 256], F32, tag="cen", name="cen")
            nc.vector.tensor_scalar_sub(cen[:], hb[:], mu[:])
            acc2 = wk.tile([16, 1], F32, tag="acc2", name="acc2")
            nc.scalar.activation(dum[:], cen[:], mybir.ActivationFunctionType.Square,
                                 bias=zb[0:16, :], accum_out=acc2[:])
            sd = wk.tile([16, 1], F32, tag="sd", name="sd")
            nc.scalar.activation(sd[:], acc2[:], mybir.ActivationFunctionType.Sqrt,
                                 scale=1.0 / H, bias=eps16[:])
            rstd = wk.tile([16, 1], F32, tag="rstd", name="rstd")
            nc.vector.reciprocal(rstd[:], sd[:])
            nrm = wk.tile([16, 256], F32, tag="nrm", name="nrm")
            nc.vector.tensor_scalar_mul(nrm[:], cen[:], rstd[:])
            gam = wk.tile([16, 256], F32, tag="gam", name="gam")
            nc.sync.dma_start(gam[:], gam_d[:])
            bet = wk.tile([16, 256], F32, tag="bet", name="bet")
            nc.sync.dma_start(bet[:], bet_d[:])
            nc.vector.tensor_mul(nrm[:], nrm[:], gam[:])
            out = wk.tile([16, 256], F32, tag="out", name="out")
            nc.vector.tensor_add(out[:], nrm[:], bet[:])
            nc.sync.dma_start(y_d[:], out[:])

    _split_excess_waits(nc)
    return nc


def prep_inputs(x, W_ih, W_hh, b_ih, b_hh, ln_gamma, ln_beta, t_steps=T):
    """host-side shard + transpose + cast. Returns per-core input dicts."""
    bf = ml_dtypes.bfloat16
    # scale the g-gate rows (PERM'd rows 768:1024) by 2: tanh(g)=2*sig(2g)-1
    gscale = np.ones((G4, 1), np.float32)
    gscale[3 * H:] = 2.0
    wih_p = W_ih[:, PERM, :] * gscale
    whh_p = W_hh[:, PERM, :] * gscale
    bias_p = (b_ih + b_hh)[:, PERM] * gscale[:, 0]
    wih = np.ascontiguousarray(np.transpose(wih_p, (0, 2, 1))).reshape(NUM_LAYERS, 2, 128, G4)
    whh = np.ascontiguousarray(np.transpose(whh_p, (0, 2, 1))).reshape(NUM_LAYERS, 2, 128, G4)
    biasm = bias_p.reshape(NUM_LAYERS, 2, 4, 128)
    ind = np.zeros((4, 512), np.float32)
    for k in range(4):
        ind[k, k * 128:(k + 1) * 128] = 1.0
    ident = np.eye(128, dtype=np.float32)
    ins = []
    for cid in range(N_CORES):
        xs = x[cid * BL:(cid + 1) * BL, :t_steps, :]        # [16, t, 256]
        xtp = np.transpose(xs, (2, 1, 0)).reshape(F, t_steps * BL)  # [256, t*16]
        ins.append({
            "xt": np.ascontiguousarray(xtp.reshape(2, 128, t_steps * BL)).astype(bf),
            "wih": wih.astype(bf), "whh": whh.astype(bf),
            "biasm": biasm.astype(bf), "ind": ind.astype(bf),
            "ident": ident,
            "gam": np.broadcast_to(ln_gamma, (BL, H)).astype(np.float32).copy(),
            "bet": np.broadcast_to(ln_beta, (BL, H)).astype(np.float32).copy(),
        })
    return ins


_CACHED = {}


def kernel(x, W_ih, W_hh, b_ih, b_hh, ln_gamma, ln_beta):
    from concourse.bass_utils import run_bass_kernel_spmd
    x = np.asarray(x, dtype=np.float32)
    ins = prep_inputs(np.asarray(x), np.asarray(W_ih), np.asarray(W_hh),
                      np.asarray(b_ih), np.asarray(b_hh),
                      np.asarray(ln_gamma), np.asarray(ln_beta))
    if "nc" not in _CACHED:
        _CACHED["nc"] = build(T)
    res = run_bass_kernel_spmd(_CACHED["nc"], ins, core_ids=list(range(N_CORES)))
    return np.concatenate([res.results[c]["y"] for c in range(N_CORES)], axis=0)


# revision 16
# speedup vs baseline: 1.3342x; 1.0011x over previous
"""2-layer LSTM (B=128, T=1024, H=256) + last-step LayerNorm on 8 trn2 cores.

Data-parallel over batch (16 rows/core). Per core, everything is kept in a
transposed layout (hidden/gate dims on partitions, batch on the free axis):

  - gates.T for a block of 8 timesteps live in one PSUM tile [128, 1024] per
    layer (col = 128*chunk + 16*step_in_block + b). Each block's tile is
    initialized by two N=512 "indicator" matmuls that write the bias to the
    whole bank (start=True clears has_written), then the x-projection
    (Wih.T chunks stationary, x.T streaming) and per-step recurrent terms
    (Whh.T chunks stationary fp8, h.T streaming) accumulate on top.
  - the per-step chain is kept short: ONE sigmoid per layer covers all four
    gates (g rows are pre-scaled by 2 on the host so tanh(g) = 2*sig(2g)-1;
    the affine is a single fused tensor_scalar on DVE), then c/h updates are
    short [128, 32] vector ops, then tanh(c) and the h write (bf16, where
    the next matmul streams it from).
  - layer 2 runs 8 steps behind layer 1; per-wave issue order anti-phases
    the two chains across the PE/ACT/DVE FIFOs so one chain's matmul burst
    overlaps the other's activation/vector stages.
  - all operands bf16 (fp8 Whh was tried: no speedup -- the recurrent burst
    is MM-issue-floor-bound at ~27ns/tile either way -- so bf16 keeps the
    better numerics), accumulation fp32, c stays fp32.

Final step: PE transpose of h2 back to [16, 256], LayerNorm, DMA out.
"""
import sys

sys.path.insert(0, "/opt/trn_rl_repo")

import numpy as np
import ml_dtypes

import concourse.bass as bass
import concourse.mybir as mybir
import concourse.tile as tile

NUM_LAYERS = 2
H = 256
F = 256
B, T = 128, 1024
LN_EPS = 1e-5
N_CORES = 8
BL = B // N_CORES          # batch rows per core = 16
G4 = 4 * H                 # 1024 gate dims
NCH = G4 // 128            # 8 chunks of gate dims
SBLK = 8                   # timesteps per x-projection block
BF16 = mybir.dt.bfloat16
F32 = mybir.dt.float32

# gate reorder: torch (i,f,g,o) -> (i,f,o,g)
PERM = np.concatenate([np.arange(0, 2 * H), np.arange(3 * H, 4 * H),
                       np.arange(2 * H, 3 * H)])


def _split_excess_waits(nc):
    """walrus in this container rejects instructions with >1 sem wait
    (CoreV3 setupSyncWait). Hoist excess waits onto NoOps just before."""
    for fn in nc.m.functions:
        for blk in fn.blocks:
            insts = list(blk.instructions)
            out, n_new = [], 0
            for inst in insts:
                si = inst.sync_info
                waits = list(si.on_wait) if si is not None else []
                if len(waits) > 1:
                    head, rest = waits[:-1], waits[-1:]
                    for wt in head:
                        nop = mybir.InstNoOp(
                            name=f"{inst.name}-ws{n_new}",
                            engine=inst.engine,
                            ins=[], outs=[],
                            sync_info=mybir.SyncInfo(on_wait=[wt], on_update=[]),
                        )
                        n_new += 1
                        out.append(nop)
                    inst.sync_info = mybir.SyncInfo(
                        on_wait=rest, on_update=list(si.on_update))
                out.append(inst)
            if n_new:
                try:
                    blk.instructions = out
                except Exception:
                    blk.set_instructions(out)


def build(t_steps=T):
    nc = bass.Bass()
    TB = t_steps * BL
    xt_d = nc.dram_tensor("xt", [2, 128, TB], BF16, kind="ExternalInput")
    wih_d = nc.dram_tensor("wih", [NUM_LAYERS, 2, 128, G4], BF16, kind="ExternalInput")
    whh_d = nc.dram_tensor("whh", [NUM_LAYERS, 2, 128, G4], BF16, kind="ExternalInput")
    biasm_d = nc.dram_tensor("biasm", [NUM_LAYERS, 2, 4, 128], BF16, kind="ExternalInput")
    ind_d = nc.dram_tensor("ind", [4, 512], BF16, kind="ExternalInput")
    ident_d = nc.dram_tensor("ident", [128, 128], F32, kind="ExternalInput")
    gam_d = nc.dram_tensor("gam", [BL, H], F32, kind="ExternalInput")
    bet_d = nc.dram_tensor("bet", [BL, H], F32, kind="ExternalInput")
    y_d = nc.dram_tensor("y", [BL, H], F32, kind="ExternalOutput")

    NB = t_steps // SBLK
    with tile.TileContext(nc) as tc:
        with (
            tc.tile_pool(name="wts", bufs=1) as wts,
            tc.tile_pool(name="state", bufs=1) as st,
            tc.tile_pool(name="work", bufs=4) as wk,
            tc.tile_pool(name="psum", bufs=2, space="PSUM") as ps,
        ):
            # resident tensors (partition dim first on every SBUF tile)
            xt = [wts.tile([128, TB], BF16, tag=f"xt{kw}", name=f"xt{kw}") for kw in (0, 1)]
            for kw in (0, 1):
                nc.sync.dma_start(xt[kw][:], xt_d[kw])
            wih = [[wts.tile([128, G4], BF16, tag=f"wih{l}{kw}", name=f"wih{l}{kw}") for kw in (0, 1)]
                   for l in range(NUM_LAYERS)]
            whh = [[wts.tile([128, G4], BF16, tag=f"whh{l}{kw}", name=f"whh{l}{kw}") for kw in (0, 1)]
                   for l in range(NUM_LAYERS)]
            biasm = [[wts.tile([4, 128], BF16, tag=f"bm{l}{b}", name=f"bm{l}{b}") for b in (0, 1)]
                     for l in range(NUM_LAYERS)]
            for l in range(NUM_LAYERS):
                for kw in (0, 1):
                    nc.sync.dma_start(wih[l][kw][:], wih_d[l, kw])
                    nc.sync.dma_start(whh[l][kw][:], whh_d[l, kw])
                for b in (0, 1):
                    nc.sync.dma_start(biasm[l][b][:], biasm_d[l, b])
            ind = wts.tile([4, 512], BF16, tag="ind", name="ind")
            nc.sync.dma_start(ind[:], ind_d[:])
            ident = wts.tile([128, 128], F32, tag="ident", name="ident")
            nc.sync.dma_start(ident[:], ident_d[:])
            zb = wts.tile([128, 1], F32, tag="zb", name="zb")
            nc.vector.memset(zb[:], 0.0)
            eps16 = wts.tile([16, 1], F32, tag="eps16", name="eps16")
            nc.vector.memset(eps16[:], LN_EPS)

            # persistent state
            hist = st.tile([128, SBLK * 32], BF16, tag="hist", name="hist")   # layer-1 h ring
            h2 = st.tile([128, 32], BF16, tag="h2", name="h2")
            c1 = st.tile([128, 32], F32, tag="c1", name="c1")
            c2 = st.tile([128, 32], F32, tag="c2", name="c2")
            h2f = st.tile([128, 32], F32, tag="h2f", name="h2f")

            xp_cur = [None, None]   # current psum block tile per layer
            xp_next0 = [None]       # layer-0 tile being prepped for next block
            xp_next1 = [None]       # layer-1 tile being prepped for next block

            def new_tile(l):
                return ps.tile([128, NCH * 128], F32, tag=f"xp{l}", name=f"xp{l}")

            def bias_mms(l, xp, banks=(0, 1)):
                for b in banks:
                    nc.tensor.matmul(
                        xp[:, b * 512:(b + 1) * 512], biasm[l][b][:], ind[:],
                        start=True, stop=False, skip_group_check=True)

            def xproj_mms(l, k, chunks):
                """x-projection matmuls (all slots) for block k of layer 0."""
                xp = xp_next0[0]
                rhs = [xt[kw][:, k * SBLK * BL:(k + 1) * SBLK * BL] for kw in (0, 1)]
                for ch in chunks:
                    o = xp[:, ch * 128:(ch + 1) * 128]
                    for kw in (0, 1):
                        nc.tensor.matmul(
                            o, wih[l][kw][:, ch * 128:(ch + 1) * 128], rhs[kw],
                            start=False, stop=False, skip_group_check=True)

            def xproj1_mms(xp, s0, s1, chunks=range(NCH)):
                """layer-1 x-projection for ring slots [s0, s1)."""
                hv = hist[:].rearrange("p (s w) -> p s w", s=SBLK)
                rhs = [hv[:, s0:s1, 0:BL], hv[:, s0:s1, BL:2 * BL]]
                for ch in chunks:
                    o = xp[:, ch * 128 + s0 * 16: ch * 128 + s1 * 16]
                    for kw in (0, 1):
                        nc.tensor.matmul(
                            o, wih[1][kw][:, ch * 128:(ch + 1) * 128], rhs[kw],
                            start=False, stop=False, skip_group_check=True)

            def rec_mms(l, t):
                """recurrent matmuls for one step (all 8 chunks x 2 kw)."""
                if t == 0:
                    return
                s = t % SBLK
                xp = xp_cur[l]
                if l == 0:
                    hsrc = hist[:, ((t - 1) % SBLK) * 32:((t - 1) % SBLK) * 32 + 32]
                else:
                    hsrc = h2[:]
                for ch in range(NCH):
                    o = xp[:, ch * 128 + 16 * s: ch * 128 + 16 * s + 16]
                    for kw in (0, 1):
                        nc.tensor.matmul(
                            o, whh[l][kw][:, ch * 128:(ch + 1) * 128],
                            hsrc[:, 16 * kw:16 * kw + 16],
                            start=False, stop=(kw == 1), skip_group_check=True)

            sig_t = [None, None]
            tg_t = [None, None]
            ig_t = [None, None]
            fc_t = [None, None]
            tc_t = [None, None]

            def act_sig(l, t):
                """one sigmoid over all 4 gates (g pre-scaled x2 on host)."""
                s = t % SBLK
                xpv = xp_cur[l][:].rearrange("p (c s w) -> p c s w", c=NCH, s=SBLK)
                sig = wk.tile([128, 128], F32, tag=f"sig{l}", name=f"sig{l}")
                sig_t[l] = sig
                nc.scalar.activation(sig[:].rearrange("p (c w) -> p c w", c=NCH),
                                     xpv[:, 0:NCH, s, :],
                                     mybir.ActivationFunctionType.Sigmoid)

            def dve_c(l, t):
                """tanh(g) affine + c update."""
                sig = sig_t[l]
                c_t = c1 if l == 0 else c2
                tg = wk.tile([128, 32], F32, tag=f"tg{l}", name=f"tg{l}")
                tg_t[l] = tg
                nc.vector.tensor_scalar(tg[:], sig[:, 96:128], 2.0, 1.0,
                                        mybir.AluOpType.mult,
                                        mybir.AluOpType.subtract)
                if t > 0:
                    fc = wk.tile([128, 32], F32, tag=f"fc{l}", name=f"fc{l}")
                    fc_t[l] = fc
                    nc.vector.tensor_mul(fc[:], sig[:, 32:64], c_t[:])
                ig = wk.tile([128, 32], F32, tag=f"ig{l}", name=f"ig{l}")
                ig_t[l] = ig
                nc.vector.tensor_mul(ig[:], sig[:, 0:32], tg[:])
                if t > 0:
                    nc.vector.tensor_add(c_t[:], ig[:], fc_t[l][:])
                else:
                    nc.vector.tensor_copy(c_t[:], ig[:])

            def act_tc(l, t):
                c_t = c1 if l == 0 else c2
                tc_ = wk.tile([128, 32], F32, tag=f"tc{l}", name=f"tc{l}")
                tc_t[l] = tc_
                nc.scalar.activation(tc_[:], c_t[:],
                                     mybir.ActivationFunctionType.Tanh)

            def dve_h(l, t):
                sig = sig_t[l]
                if l == 0:
                    hdst = hist[:, (t % SBLK) * 32:(t % SBLK) * 32 + 32]
                else:
                    hdst = h2[:]
                nc.vector.tensor_mul(hdst, sig[:, 64:96], tc_t[l][:])
                if l == 1 and t == t_steps - 1:
                    nc.vector.tensor_mul(h2f[:], sig[:, 64:96], tc_t[l][:])

            # ---- main wave loop: wave w runs l0 step w and l1 step w-8
            for w in range(t_steps + SBLK):
                t0 = w if w < t_steps else None                  # layer-0 step
                t1 = w - SBLK if w >= SBLK else None             # layer-1 step
                s = w % SBLK

                # block-boundary tile switches (prep happened in prior waves)
                if s == 0:
                    if t1 is not None:
                        xp_cur[1] = xp_next1[0]
                        xp_next1[0] = None
                    if t0 is not None:
                        if w == 0:
                            xp_next0[0] = new_tile(0)
                            bias_mms(0, xp_next0[0])
                            xproj_mms(0, 0, range(NCH))
                        xp_cur[0] = xp_next0[0]
                        xp_next0[0] = None

                # recurrent bursts: l1 first (its chain leads the wave)
                if t1 is not None:
                    rec_mms(1, t1)
                if t0 is not None:
                    rec_mms(0, t0)

                # prep work, evened out: every wave carries ~2 small (4-MM)
                # layer-1 xproj pieces with >=1 wave of data slack, and at
                # most one fat (N=512, ~600ns) bias matmul, so no wave's PE
                # prep tail can push a stalled recurrent burst far.
                # layer-1 pieces for ring-slot pairs of the CURRENT tile
                # ({4,5} and {6,7} complete after the block switch):
                if t1 is not None:
                    if s == 0:
                        xproj1_mms(xp_cur[1], 4, 6, range(0, 4))
                    elif s == 1:
                        xproj1_mms(xp_cur[1], 4, 6, range(4, NCH))
                    elif s == 2:
                        xproj1_mms(xp_cur[1], 6, 8, range(0, 4))
                    elif s == 3:
                        xproj1_mms(xp_cur[1], 6, 8, range(4, NCH))
                # layer-1 tile for the next block: bias banks at s=3,4, then
                # slot-pair xproj pieces one wave after their h1 data lands
                k1 = w // SBLK
                if k1 < NB:
                    if s == 3:
                        xp_next1[0] = new_tile(1)
                        bias_mms(1, xp_next1[0], (0,))
                    elif s == 4:
                        bias_mms(1, xp_next1[0], (1,))
                        xproj1_mms(xp_next1[0], 0, 2, range(0, 4))
                    elif s == 5:
                        xproj1_mms(xp_next1[0], 0, 2, range(4, NCH))
                    elif s == 6:
                        xproj1_mms(xp_next1[0], 2, 4, range(0, 4))
                    elif s == 7:
                        xproj1_mms(xp_next1[0], 2, 4, range(4, NCH))
                # layer-0 prep for the next block (x resident; free order)
                if t0 is not None and t0 // SBLK + 1 < NB:
                    nk = t0 // SBLK + 1
                    if s == 1:
                        xp_next0[0] = new_tile(0)
                        bias_mms(0, xp_next0[0], (0,))
                    elif s == 2:
                        bias_mms(0, xp_next0[0], (1,))
                        xproj_mms(0, nk, (0, 1))
                    elif s == 3:
                        xproj_mms(0, nk, (2, 3))
                    elif s == 4:
                        xproj_mms(0, nk, (4, 5))
                    elif s == 5:
                        xproj_mms(0, nk, (6, 7))

                # anti-phased post-matmul stages
                if t1 is not None:
                    act_sig(1, t1)
                    dve_c(1, t1)
                if t0 is not None:
                    act_sig(0, t0)
                if t1 is not None:
                    act_tc(1, t1)
                if t0 is not None:
                    dve_c(0, t0)
                if t1 is not None:
                    dve_h(1, t1)
                if t0 is not None:
                    act_tc(0, t0)
                    dve_h(0, t0)

            # ---- LayerNorm over H on h2f (h2.T layout) -> y [16, 256]
            pt = ps.tile([16, 256], F32, tag="xp0", name="pt")
            nc.tensor.transpose(pt[:, 0:128], h2f[:, 0:16], ident[:])
            nc.tensor.transpose(pt[:, 128:256], h2f[:, 16:32], ident[:])
            hb = wk.tile([16, 256], F32, tag="hb", name="hb")
            nc.vector.tensor_copy(hb[:], pt[:])
            dum = wk.tile([16, 256], F32, tag="dum", name="dum")
            acc = wk.tile([16, 1], F32, tag="acc", name="acc")
            nc.scalar.activation(dum[:], hb[:], mybir.ActivationFunctionType.Copy,
                                 accum_out=acc[:])
            mu = wk.tile([16, 1], F32, tag="mu", name="mu")
            nc.vector.tensor_scalar_mul(mu[:], acc[:], 1.0 / H)
            cen = wk.tile([16, 256], F32, tag="cen", name="cen")
            nc.vector.tensor_scalar_sub(cen[:], hb[:], mu[:])
            acc2 = wk.tile([16, 1], F32, tag="acc2", name="acc2")
            nc.scalar.activation(dum[:], cen[:], mybir.ActivationFunctionType.Square,
                                 bias=zb[0:16, :], accum_out=acc2[:])
            sd = wk.tile([16, 1], F32, tag="sd", name="sd")
            nc.scalar.activation(sd[:], acc2[:], mybir.ActivationFunctionType.Sqrt,
                                 scale=1.0 / H, bias=eps16[:])
            rstd = wk.tile([16, 1], F32, tag="rstd", name="rstd")
            nc.vector.reciprocal(rstd[:], sd[:])
            nrm = wk.tile([16, 256], F32, tag="nrm", name="nrm")
            nc.vector.tensor_scalar_mul(nrm[:], cen[:], rstd[:])
            gam = wk.tile([16, 256], F32, tag="gam", name="gam")
            nc.sync.dma_start(gam[:], gam_d[:])
            bet = wk.tile([16, 256], F32, tag="bet", name="bet")
            nc.sync.dma_start(bet[:], bet_d[:])
            nc.vector.tensor_mul(nrm[:], nrm[:], gam[:])
            out = wk.tile([16, 256], F32, tag="out", name="out")
            nc.vector.tensor_add(out[:], nrm[:], bet[:])
            nc.sync.dma_start(y_d[:], out[:])

    _split_excess_waits(nc)
    return nc


def prep_inputs(x, W_ih, W_hh, b_ih, b_hh, ln_gamma, ln_beta, t_steps=T):
    """host-side shard + transpose + cast. Returns per-core input dicts."""
    bf = ml_dtypes.bfloat16
    # scale the g-gate rows (PERM'd rows 768:1024) by 2: tanh(g)=2*sig(2g)-1
    gscale = np.ones((G4, 1), np.float32)
    gscale[3 * H:] = 2.0
    wih_p = W_ih[:, PERM, :] * gscale
    whh_p = W_hh[:, PERM, :] * gscale
    bias_p = (b_ih + b_hh)[:, PERM] * gscale[:, 0]
    wih = np.ascontiguousarray(np.transpose(wih_p, (0, 2, 1))).reshape(NUM_LAYERS, 2, 128, G4)
    whh = np.ascontiguousarray(np.transpose(whh_p, (0, 2, 1))).reshape(NUM_LAYERS, 2, 128, G4)
    biasm = bias_p.reshape(NUM_LAYERS, 2, 4, 128)
    ind = np.zeros((4, 512), np.float32)
    for k in range(4):
        ind[k, k * 128:(k + 1) * 128] = 1.0
    ident = np.eye(128, dtype=np.float32)
    ins = []
    for cid in range(N_CORES):
        xs = x[cid * BL:(cid + 1) * BL, :t_steps, :]        # [16, t, 256]
        xtp = np.transpose(xs, (2, 1, 0)).reshape(F, t_steps * BL)  # [256, t*16]
        ins.append({
            "xt": np.ascontiguousarray(xtp.reshape(2, 128, t_steps * BL)).astype(bf),
            "wih": wih.astype(bf), "whh": whh.astype(bf),
            "biasm": biasm.astype(bf), "ind": ind.astype(bf),
            "ident": ident,
            "gam": np.broadcast_to(ln_gamma, (BL, H)).astype(np.float32).copy(),
            "bet": np.broadcast_to(ln_beta, (BL, H)).astype(np.float32).copy(),
        })
    return ins


_CACHED = {}


def kernel(x, W_ih, W_hh, b_ih, b_hh, ln_gamma, ln_beta):
    from concourse.bass_utils import run_bass_kernel_spmd
    x = np.asarray(x, dtype=np.float32)
    ins = prep_inputs(np.asarray(x), np.asarray(W_ih), np.asarray(W_hh),
                      np.asarray(b_ih), np.asarray(b_hh),
                      np.asarray(ln_gamma), np.asarray(ln_beta))
    if "nc" not in _CACHED:
        _CACHED["nc"] = build(T)
    res = run_bass_kernel_spmd(_CACHED["nc"], ins, core_ids=list(range(N_CORES)))
    return np.concatenate([res.results[c]["y"] for c in range(N_CORES)], axis=0)
